# revision 3
# baseline (speedup 1.0000x reference)
"""CRF NLL loss on 8 Trainium2 NeuronCores - segmented-contraction forward algorithm
with a seq_len staircase.

Extends kernel2's segmented-contraction scheme: batch elements are sorted by
seq_len and dealt strided across cores, so each core's 64 columns (sorted
ascending by L) span the length distribution. Pair-chain j (segments 2j,2j+1)
only processes columns c >= lo_j = max(0, 4j - M): a column with L < 2jC has
its capture in an earlier segment, so later pairs can drop it. The static
staircase has margin M; the host verifies it against the actual lengths and
computes the rare violating elements exactly in f64 (a few ms each).

Device: 2 lockstep groups (even/odd pairs, widths ~344/316 instead of 512),
one fat bf16 matmul + one wide DVE multiply per group per chain step.
"""
import os
import numpy as np
from contextlib import ExitStack
from ml_dtypes import bfloat16

import concourse.bacc as bacc
import concourse.bass as bass
import concourse.tile as tile
from concourse import mybir
from concourse.bass_utils import run_bass_kernel_spmd

B, T, K = 512, 1024, 48
START, STOP = 46, 47
NEG = -10000.0
KR = 46
HOLD = 46
KS = 47
P2 = 2 * KS
NCORES = 8
BC = B // NCORES
C = 32
W = int(os.environ.get("K_W", "6"))
S = T // C          # 32
PAIRS = S // 2      # 16
NGRP = 2
MARGIN = int(os.environ.get("K_MARGIN", "8"))

# static staircase: pair j processes columns [LO[j], BC)
LO = [max(0, 4 * j - MARGIN) for j in range(PAIRS)]
WID = [BC - lo for lo in LO]
GPAIRS = [[j for j in range(PAIRS) if j % NGRP == g] for g in range(NGRP)]
GWS = [sum(WID[j] for j in gp) for gp in GPAIRS]   # group widths
# offset of pair j inside its group tile
POFF = {}
for g, gp in enumerate(GPAIRS):
    off = 0
    for j in gp:
        POFF[j] = off
        off += WID[j]

_nc_cache = {}


def _build_module(w=W, repeat=1):
    key = (w, repeat)
    if key in _nc_cache:
        return _nc_cache[key]
    nstep = w + C
    nc = bacc.Bacc(
        "TRN2",
        target_bir_lowering=False,
        debug=False,
        enable_asserts=False,
        num_devices=NCORES,
    )
    bf16 = mybir.dt.bfloat16
    f32 = mybir.dt.float32
    e_dram = nc.dram_tensor("etil", [P2, P2], bf16, kind="ExternalInput").ap()
    g_drams = [
        nc.dram_tensor(f"gall{g}", [P2, nstep, GWS[g]], bf16,
                       kind="ExternalInput").ap()
        for g in range(NGRP)
    ]
    w_drams = [
        nc.dram_tensor(f"winit{g}", [P2, GWS[g]], bf16, kind="ExternalInput").ap()
        for g in range(NGRP)
    ]
    snap_drams = [
        nc.dram_tensor(f"snaps{g}", [P2, 3, GWS[g]], bf16,
                       kind="ExternalOutput").ap()
        for g in range(NGRP)
    ]

    CH = 8
    chunks = []
    s0 = 0
    while s0 < nstep:
        chunks.append((s0, min(CH, nstep - s0)))
        s0 += CH

    with tile.TileContext(nc) as tc:
        with ExitStack() as ctx:
            const = ctx.enter_context(tc.tile_pool(name="const", bufs=1))
            wpool = ctx.enter_context(tc.tile_pool(name="wp", bufs=2))
            gpool = ctx.enter_context(tc.tile_pool(name="gp", bufs=1))
            psum_p = ctx.enter_context(tc.tile_pool(name="ps", bufs=2, space="PSUM"))

            etile = const.tile([P2, P2], bf16)
            nc.sync.dma_start(out=etile, in_=e_dram)

            wcur = []
            for g in range(NGRP):
                wt = wpool.tile([P2, GWS[g]], bf16, tag=f"w{g}")
                nc.sync.dma_start(out=wt, in_=w_drams[g])
                wcur.append(wt)

            gt = {}
            for ci, (c0, cl) in enumerate(chunks):
                for g in range(NGRP):
                    t = gpool.tile([P2, cl, GWS[g]], bf16, tag=f"g{g}c{ci}")
                    nc.sync.dma_start(out=t, in_=g_drams[g][:, c0 : c0 + cl, :])
                    gt[(g, ci)] = t

            for rep in range(repeat):
                for i in range(nstep):
                    ci, off = divmod(i, CH)
                    for g in range(NGRP):
                        ps = psum_p.tile([P2, GWS[g]], f32, tag=f"ps{g}")
                        nc.tensor.matmul(ps, etile, wcur[g], start=True, stop=True)
                        wn = wpool.tile([P2, GWS[g]], bf16, tag=f"w{g}")
                        nc.vector.tensor_mul(wn, ps, gt[(g, ci)][:, off, :])
                        wcur[g] = wn
                        if rep == 0:
                            if i == w - 1:
                                nc.sync.dma_start(out=snap_drams[g][:, 0, :], in_=wn)
                            if i == nstep - 1:
                                nc.sync.dma_start(out=snap_drams[g][:, 1, :], in_=wn)
                            if i == C - 2 and g == 0:
                                nc.sync.dma_start(out=snap_drams[g][:, 2, :], in_=wn)

    nc.compile()
    _nc_cache[key] = nc
    return nc


def _build_timing(reps, w=W):
    """Timing-only module: the 38-step pass wrapped in a hardware For_i loop.
    Results are garbage (state carries across iterations); per-pass time =
    slope between two reps values."""
    key = ("timing", w, reps)
    if key in _nc_cache:
        return _nc_cache[key]
    nstep = w + C
    nc = bacc.Bacc(
        "TRN2",
        target_bir_lowering=False,
        debug=False,
        enable_asserts=False,
        num_devices=NCORES,
    )
    bf16 = mybir.dt.bfloat16
    f32 = mybir.dt.float32
    e_dram = nc.dram_tensor("etil", [P2, P2], bf16, kind="ExternalInput").ap()
    g_drams = [
        nc.dram_tensor(f"gall{g}", [P2, nstep, GWS[g]], bf16,
                       kind="ExternalInput").ap()
        for g in range(NGRP)
    ]
    w_drams = [
        nc.dram_tensor(f"winit{g}", [P2, GWS[g]], bf16, kind="ExternalInput").ap()
        for g in range(NGRP)
    ]
    snap_drams = [
        nc.dram_tensor(f"snaps{g}", [P2, 3, GWS[g]], bf16,
                       kind="ExternalOutput").ap()
        for g in range(NGRP)
    ]
    CH = 8
    chunks = []
    s0 = 0
    while s0 < nstep:
        chunks.append((s0, min(CH, nstep - s0)))
        s0 += CH

    with tile.TileContext(nc) as tc:
        with ExitStack() as ctx:
            const = ctx.enter_context(tc.tile_pool(name="const", bufs=1))
            wpool = ctx.enter_context(tc.tile_pool(name="wp", bufs=4))
            gpool = ctx.enter_context(tc.tile_pool(name="gp", bufs=1))
            psum_p = ctx.enter_context(tc.tile_pool(name="ps", bufs=2, space="PSUM"))

            etile = const.tile([P2, P2], bf16)
            nc.sync.dma_start(out=etile, in_=e_dram)
            wcur = []
            for g in range(NGRP):
                wt = wpool.tile([P2, GWS[g]], bf16, tag=f"w{g}")
                nc.sync.dma_start(out=wt, in_=w_drams[g])
                wcur.append(wt)
            gt = {}
            for ci, (c0, cl) in enumerate(chunks):
                for g in range(NGRP):
                    t = gpool.tile([P2, cl, GWS[g]], bf16, tag=f"g{g}c{ci}")
                    nc.sync.dma_start(out=t, in_=g_drams[g][:, c0 : c0 + cl, :])
                    gt[(g, ci)] = t

            with tc.For_i(0, reps, 1):
                for i in range(nstep):
                    ci, off = divmod(i, CH)
                    for g in range(NGRP):
                        ps = psum_p.tile([P2, GWS[g]], f32, tag=f"ps{g}")
                        nc.tensor.matmul(ps, etile, wcur[g], start=True, stop=True)
                        wn = wpool.tile([P2, GWS[g]], bf16, tag=f"w{g}")
                        nc.vector.tensor_mul(wn, ps, gt[(g, ci)][:, off, :])
                        wcur[g] = wn

            for g in range(NGRP):
                for slot in range(3):
                    nc.sync.dma_start(out=snap_drams[g][:, slot, :], in_=wcur[g])

    nc.compile()
    _nc_cache[key] = nc
    return nc


def _shifts_and_g(feats, seq_len, trans):
    mx = feats.max(axis=2)
    E64 = np.exp(trans.astype(np.float64)).T
    drift = []
    for b in range(6):
        fv = np.full(K, NEG, dtype=np.float64)
        fv[START] = 0.0
        for t in range(min(int(seq_len[b]), 256)):
            m = fv.max()
            wv = np.exp(fv - m)
            fv = np.log(E64.T @ wv + 1e-300) + m + feats[b, t]
            drift.append((fv.max() - m) - mx[b, t])
    mu = float(np.mean(drift))
    c = mx + mu
    Ccum = np.cumsum(c, axis=1, dtype=np.float64)
    C_at_L = Ccum[np.arange(B), seq_len - 1]

    taus = np.arange(1, T + 1)
    live = taus[None, :] <= seq_len[:, None]
    g = np.zeros((B, T, KS), dtype=np.float32)
    g[:, :, :KR] = np.exp(feats[:, :, :KR] - c[:, :, None]) * live[:, :, None]
    g[:, :, HOLD] = (taus[None, :] >= (seq_len[:, None] + 1)).astype(np.float32)
    return g, C_at_L


def _host_prep(feats, seq_len, trans, w=W):
    feats = np.ascontiguousarray(feats, dtype=np.float32)
    seq_len = np.asarray(seq_len, dtype=np.int64)
    trans = np.asarray(trans, dtype=np.float32)
    nstep = w + C

    g, C_at_L = _shifts_and_g(feats, seq_len, trans)

    Et = np.zeros((KS, KS), dtype=np.float32)
    Et[:KR, :KR] = np.exp(trans[:KR, :KR]).T
    Et[:KR, HOLD] = np.exp(trans[STOP, :KR])
    Et[HOLD, HOLD] = 1.0
    etil2 = np.zeros((P2, P2), dtype=np.float32)
    etil2[:KS, :KS] = Et
    etil2[KS:, KS:] = Et
    etil2 = etil2.astype(bfloat16)

    e_start = np.exp(trans[:KR, START])
    w1 = g[:, 0, :].copy()
    w1[:, :KR] *= e_start[None, :]
    w1[:, HOLD] = 0.0

    # sorted strided assignment: core k column c -> batch order[c*8+k]
    order = np.argsort(seq_len, kind="stable")
    asg = order.reshape(BC, NCORES)            # [c, core]

    # chain-step -> step table per (pair, half): 0-indexed into T axis, T=dead
    tmap = np.full((PAIRS, 2, nstep), T, dtype=np.int64)
    for j in range(PAIRS):
        for half in range(2):
            seg = 2 * j + half
            if seg == 0:
                for i in range(C - 1):
                    tmap[j, half, i] = 1 + i
            else:
                for i in range(nstep):
                    if i < w:
                        t = seg * C - w + 1 + i
                    else:
                        t = seg * C + 1 + (i - w)
                    tmap[j, half, i] = t - 1

    probe = np.zeros(KS, dtype=np.float32)
    probe[:KR] = 1.0 / KR

    gpad = np.concatenate([g, np.zeros((B, 1, KS), np.float32)], axis=1)
    galls, winits = [], []
    for cix in range(NCORES):
        bidx = asg[:, cix]                     # [BC] batch ids, ascending L
        sub = gpad[bidx]                       # [BC, T+1, KS]
        ga = [np.zeros((P2, nstep, GWS[gi]), dtype=np.float32) for gi in range(NGRP)]
        wi = [np.zeros((P2, GWS[gi]), dtype=np.float32) for gi in range(NGRP)]
        for j in range(PAIRS):
            gi, off, wd, lo = j % NGRP, POFF[j], WID[j], LO[j]
            for half in range(2):
                rows = slice(half * KS, (half + 1) * KS)
                # emis [wd_cols, nstep, KS] -> [KS, nstep, wd]
                em = sub[lo:, tmap[j, half], :]
                ga[gi][rows, :, off : off + wd] = em.transpose(2, 1, 0)
            wi[gi][0:KS, off : off + wd] = probe[:, None]
            wi[gi][KS:, off : off + wd] = probe[:, None]
            if j == 0:
                wi[gi][0:KS, off : off + wd] = w1[bidx[lo:]].T
        galls.append([np.ascontiguousarray(a.astype(bfloat16)) for a in ga])
        winits.append([np.ascontiguousarray(a.astype(bfloat16)) for a in wi])

    return etil2, galls, winits, C_at_L, g, w1, asg


def _exact_logZ(feats, seq_len, trans, b):
    E64 = np.exp(trans.astype(np.float64)).T
    fv = np.full(K, NEG, dtype=np.float64)
    fv[START] = 0.0
    for t in range(int(seq_len[b])):
        m = fv.max()
        wv = np.exp(fv - m)
        fv = np.log(E64.T @ wv + 1e-300) + m + feats[b, t].astype(np.float64)
    m = fv.max()
    return float(
        np.log(np.exp(fv - m + trans[STOP, :].astype(np.float64)).sum()) + m
    )


def _gold_score(feats, tags, seq_len, trans):
    feats = np.asarray(feats, dtype=np.float32)
    tags = np.asarray(tags, dtype=np.int64)
    seq_len = np.asarray(seq_len, dtype=np.int64)
    trans = np.asarray(trans, dtype=np.float32)
    tags_ext = np.concatenate(
        [np.full((B, 1), START, dtype=np.int64), tags], axis=1
    )
    trans_sc = trans[tags_ext[:, 1:], tags_ext[:, :-1]]
    emit_sc = np.take_along_axis(feats, tags_ext[:, 1:, None], axis=2)[..., 0]
    mask = np.arange(T)[None, :] < seq_len[:, None]
    last_tag = np.take_along_axis(tags_ext, seq_len[:, None], axis=1)[:, 0]
    return (
        np.where(mask, trans_sc + emit_sc, 0.0).sum(1, dtype=np.float64)
        + trans[STOP, last_tag]
    )


def _combine(snaps_list, feats, seq_len, trans, C_at_L, w1, asg):
    """snaps_list: per-core list of [94, 3, GWS[g]] arrays per group."""
    seq_len = np.asarray(seq_len, dtype=np.int64)
    trans = np.asarray(trans, dtype=np.float32)
    qs = np.zeros((S, B, KS), dtype=np.float64)
    rs = np.zeros((S, B, KS), dtype=np.float64)
    for cix in range(NCORES):
        bidx = asg[:, cix]
        sn = [np.asarray(a).astype(np.float32) for a in snaps_list[cix]]
        for j in range(PAIRS):
            gi, off, wd, lo = j % NGRP, POFF[j], WID[j], LO[j]
            cols = bidx[lo:]
            for half in range(2):
                seg = 2 * j + half
                rows = slice(half * KS, (half + 1) * KS)
                qs[seg, cols] = sn[gi][rows, 0, off : off + wd].T
                rs[seg, cols] = sn[gi][rows, 1, off : off + wd].T
            if j == 0:
                rs[0, cols] = sn[gi][0:KS, 2, off : off + wd].T
    qs[0] = w1.astype(np.float64)

    def n1(v):
        return v[..., :KR].sum(axis=-1)

    lk = np.zeros(B)
    logk = np.zeros((S, B))
    with np.errstate(divide="ignore", invalid="ignore"):
        for s in range(1, S):
            lk = lk + np.log(n1(rs[s - 1])) - np.log(n1(qs[s]))
            logk[s] = lk
    s_cap = np.minimum(seq_len // C, S - 1)
    cap = rs[s_cap, np.arange(B), HOLD]
    full = seq_len == T
    fdot = (
        rs[S - 1][:, :KR] * np.exp(trans[STOP, :KR].astype(np.float64))[None, :]
    ).sum(axis=1)
    cap = np.where(full, fdot, cap)
    with np.errstate(divide="ignore", invalid="ignore"):
        logZ = np.log(cap) + logk[s_cap, np.arange(B)] + C_at_L

    # staircase violations: column c of core k excluded from pair j although
    # its capture chain needs it (L >= 2jC). Sorted columns make this a
    # boundary check; recompute those elements exactly on the host.
    feats32 = np.asarray(feats, dtype=np.float32)
    patched = 0
    for cix in range(NCORES):
        bidx = asg[:, cix]
        L = seq_len[bidx]
        for j in range(PAIRS):
            lo = LO[j]
            bad = np.nonzero(L[:lo] >= 2 * j * C)[0]
            for c in bad:
                b = int(bidx[c])
                logZ[b] = _exact_logZ(feats32, seq_len, trans, b)
                patched += 1
    bad = ~np.isfinite(logZ)
    for b in np.nonzero(bad)[0]:
        logZ[b] = _exact_logZ(feats32, seq_len, trans, b)
        patched += 1
    return logZ, patched


def kernel(feats, tags, seq_len, transitions):
    feats = np.asarray(feats)
    etil2, galls, winits, C_at_L, g, w1, asg = _host_prep(
        feats, seq_len, transitions
    )
    nc = _build_module()
    in_maps = []
    for cix in range(NCORES):
        m = {"etil": etil2}
        for gi in range(NGRP):
            m[f"gall{gi}"] = galls[cix][gi]
            m[f"winit{gi}"] = winits[cix][gi]
        in_maps.append(m)
    res = run_bass_kernel_spmd(nc, in_maps, list(range(NCORES)))
    snaps_list = [
        [res.results[cix][f"snaps{gi}"] for gi in range(NGRP)]
        for cix in range(NCORES)
    ]
    logZ, _ = _combine(snaps_list, feats, seq_len, transitions, C_at_L, w1, asg)
    gold = _gold_score(feats, tags, seq_len, transitions)
    return np.float32(np.mean(logZ - gold))


# revision 4
# speedup vs baseline: 1.4567x; 1.4567x over previous
"""CRF NLL loss on 8 Trainium2 NeuronCores - segmented-contraction forward algorithm
with a seq_len staircase.

Extends kernel2's segmented-contraction scheme: batch elements are sorted by
seq_len and dealt strided across cores, so each core's 64 columns (sorted
ascending by L) span the length distribution. Pair-chain j (segments 2j,2j+1)
only processes columns c >= lo_j = max(0, 4j - M): a column with L < 2jC has
its capture in an earlier segment, so later pairs can drop it. The static
staircase has margin M; the host verifies it against the actual lengths and
computes the rare violating elements exactly in f64 (a few ms each).

Device: 2 lockstep groups (even/odd pairs, widths ~344/316 instead of 512),
one fat bf16 matmul + one wide DVE multiply per group per chain step.
"""
import os
import numpy as np
from contextlib import ExitStack
from ml_dtypes import bfloat16

import concourse.bacc as bacc
import concourse.bass as bass
import concourse.tile as tile
from concourse import mybir
from concourse.bass_utils import run_bass_kernel_spmd

B, T, K = 512, 1024, 48
START, STOP = 46, 47
NEG = -10000.0
KR = 46
HOLD = 46
KS = 47
P2 = 2 * KS
NCORES = 8
BC = B // NCORES
C = 32
W = int(os.environ.get("K_W", "5"))
S = T // C          # 32
PAIRS = S // 2      # 16
NGRP = 2
MARGIN = int(os.environ.get("K_MARGIN", "1"))

# static staircase: pair j processes columns [LO[j], BC)
LO = [max(0, 4 * j - MARGIN) for j in range(PAIRS)]
WID = [BC - lo for lo in LO]
GPAIRS = [[j for j in range(PAIRS) if j % NGRP == g] for g in range(NGRP)]
GWS = [sum(WID[j] for j in gp) for gp in GPAIRS]   # group widths
# offset of pair j inside its group tile
POFF = {}
for g, gp in enumerate(GPAIRS):
    off = 0
    for j in gp:
        POFF[j] = off
        off += WID[j]

_nc_cache = {}


def _build_module(w=W, repeat=1):
    key = (w, repeat)
    if key in _nc_cache:
        return _nc_cache[key]
    nstep = w + C
    nc = bacc.Bacc(
        "TRN2",
        target_bir_lowering=False,
        debug=False,
        enable_asserts=False,
        num_devices=NCORES,
    )
    bf16 = mybir.dt.bfloat16
    f32 = mybir.dt.float32
    e_dram = nc.dram_tensor("etil", [P2, P2], bf16, kind="ExternalInput").ap()
    g_drams = [
        nc.dram_tensor(f"gall{g}", [P2, nstep, GWS[g]], bf16,
                       kind="ExternalInput").ap()
        for g in range(NGRP)
    ]
    w_drams = [
        nc.dram_tensor(f"winit{g}", [P2, GWS[g]], bf16, kind="ExternalInput").ap()
        for g in range(NGRP)
    ]
    snap_drams = [
        nc.dram_tensor(f"snaps{g}", [P2, 3, GWS[g]], bf16,
                       kind="ExternalOutput").ap()
        for g in range(NGRP)
    ]

    CH = 8
    chunks = []
    s0 = 0
    while s0 < nstep:
        chunks.append((s0, min(CH, nstep - s0)))
        s0 += CH

    with tile.TileContext(nc) as tc:
        with ExitStack() as ctx:
            const = ctx.enter_context(tc.tile_pool(name="const", bufs=1))
            wpool = ctx.enter_context(tc.tile_pool(name="wp", bufs=2))
            gpool = ctx.enter_context(tc.tile_pool(name="gp", bufs=1))
            psum_p = ctx.enter_context(tc.tile_pool(name="ps", bufs=2, space="PSUM"))

            etile = const.tile([P2, P2], bf16)
            nc.sync.dma_start(out=etile, in_=e_dram)

            wcur = []
            for g in range(NGRP):
                wt = wpool.tile([P2, GWS[g]], bf16, tag=f"w{g}")
                nc.sync.dma_start(out=wt, in_=w_drams[g])
                wcur.append(wt)

            gt = {}
            for ci, (c0, cl) in enumerate(chunks):
                for g in range(NGRP):
                    t = gpool.tile([P2, cl, GWS[g]], bf16, tag=f"g{g}c{ci}")
                    nc.sync.dma_start(out=t, in_=g_drams[g][:, c0 : c0 + cl, :])
                    gt[(g, ci)] = t

            for rep in range(repeat):
                for i in range(nstep):
                    ci, off = divmod(i, CH)
                    for g in range(NGRP):
                        ps = psum_p.tile([P2, GWS[g]], f32, tag=f"ps{g}")
                        nc.tensor.matmul(ps, etile, wcur[g], start=True, stop=True)
                        wn = wpool.tile([P2, GWS[g]], bf16, tag=f"w{g}")
                        nc.vector.tensor_mul(wn, ps, gt[(g, ci)][:, off, :])
                        wcur[g] = wn
                        if rep == 0:
                            if i == w - 1:
                                nc.sync.dma_start(out=snap_drams[g][:, 0, :], in_=wn)
                            if i == nstep - 1:
                                nc.sync.dma_start(out=snap_drams[g][:, 1, :], in_=wn)
                            if i == C - 2 and g == 0:
                                nc.sync.dma_start(out=snap_drams[g][:, 2, :], in_=wn)

    nc.compile()
    _nc_cache[key] = nc
    return nc


def _build_timing(reps, w=W):
    """Timing-only module: the 38-step pass wrapped in a hardware For_i loop.
    Results are garbage (state carries across iterations); per-pass time =
    slope between two reps values."""
    key = ("timing", w, reps)
    if key in _nc_cache:
        return _nc_cache[key]
    nstep = w + C
    nc = bacc.Bacc(
        "TRN2",
        target_bir_lowering=False,
        debug=False,
        enable_asserts=False,
        num_devices=NCORES,
    )
    bf16 = mybir.dt.bfloat16
    f32 = mybir.dt.float32
    e_dram = nc.dram_tensor("etil", [P2, P2], bf16, kind="ExternalInput").ap()
    g_drams = [
        nc.dram_tensor(f"gall{g}", [P2, nstep, GWS[g]], bf16,
                       kind="ExternalInput").ap()
        for g in range(NGRP)
    ]
    w_drams = [
        nc.dram_tensor(f"winit{g}", [P2, GWS[g]], bf16, kind="ExternalInput").ap()
        for g in range(NGRP)
    ]
    snap_drams = [
        nc.dram_tensor(f"snaps{g}", [P2, 3, GWS[g]], bf16,
                       kind="ExternalOutput").ap()
        for g in range(NGRP)
    ]
    CH = 8
    chunks = []
    s0 = 0
    while s0 < nstep:
        chunks.append((s0, min(CH, nstep - s0)))
        s0 += CH

    with tile.TileContext(nc) as tc:
        with ExitStack() as ctx:
            const = ctx.enter_context(tc.tile_pool(name="const", bufs=1))
            wpool = ctx.enter_context(tc.tile_pool(name="wp", bufs=4))
            gpool = ctx.enter_context(tc.tile_pool(name="gp", bufs=1))
            psum_p = ctx.enter_context(tc.tile_pool(name="ps", bufs=2, space="PSUM"))

            etile = const.tile([P2, P2], bf16)
            nc.sync.dma_start(out=etile, in_=e_dram)
            wcur = []
            for g in range(NGRP):
                wt = wpool.tile([P2, GWS[g]], bf16, tag=f"w{g}")
                nc.sync.dma_start(out=wt, in_=w_drams[g])
                wcur.append(wt)
            gt = {}
            for ci, (c0, cl) in enumerate(chunks):
                for g in range(NGRP):
                    t = gpool.tile([P2, cl, GWS[g]], bf16, tag=f"g{g}c{ci}")
                    nc.sync.dma_start(out=t, in_=g_drams[g][:, c0 : c0 + cl, :])
                    gt[(g, ci)] = t

            with tc.For_i(0, reps, 1):
                for i in range(nstep):
                    ci, off = divmod(i, CH)
                    for g in range(NGRP):
                        ps = psum_p.tile([P2, GWS[g]], f32, tag=f"ps{g}")
                        nc.tensor.matmul(ps, etile, wcur[g], start=True, stop=True)
                        wn = wpool.tile([P2, GWS[g]], bf16, tag=f"w{g}")
                        nc.vector.tensor_mul(wn, ps, gt[(g, ci)][:, off, :])
                        wcur[g] = wn

            for g in range(NGRP):
                for slot in range(3):
                    nc.sync.dma_start(out=snap_drams[g][:, slot, :], in_=wcur[g])

    nc.compile()
    _nc_cache[key] = nc
    return nc


def _shifts_and_g(feats, seq_len, trans):
    mx = feats.max(axis=2)
    E64 = np.exp(trans.astype(np.float64)).T
    drift = []
    for b in range(6):
        fv = np.full(K, NEG, dtype=np.float64)
        fv[START] = 0.0
        for t in range(min(int(seq_len[b]), 256)):
            m = fv.max()
            wv = np.exp(fv - m)
            fv = np.log(E64.T @ wv + 1e-300) + m + feats[b, t]
            drift.append((fv.max() - m) - mx[b, t])
    mu = float(np.mean(drift))
    c = mx + mu
    Ccum = np.cumsum(c, axis=1, dtype=np.float64)
    C_at_L = Ccum[np.arange(B), seq_len - 1]

    taus = np.arange(1, T + 1)
    live = taus[None, :] <= seq_len[:, None]
    g = np.zeros((B, T, KS), dtype=np.float32)
    g[:, :, :KR] = np.exp(feats[:, :, :KR] - c[:, :, None]) * live[:, :, None]
    g[:, :, HOLD] = (taus[None, :] >= (seq_len[:, None] + 1)).astype(np.float32)
    return g, C_at_L


def _host_prep(feats, seq_len, trans, w=W):
    feats = np.ascontiguousarray(feats, dtype=np.float32)
    seq_len = np.asarray(seq_len, dtype=np.int64)
    trans = np.asarray(trans, dtype=np.float32)
    nstep = w + C

    g, C_at_L = _shifts_and_g(feats, seq_len, trans)

    Et = np.zeros((KS, KS), dtype=np.float32)
    Et[:KR, :KR] = np.exp(trans[:KR, :KR]).T
    Et[:KR, HOLD] = np.exp(trans[STOP, :KR])
    Et[HOLD, HOLD] = 1.0
    etil2 = np.zeros((P2, P2), dtype=np.float32)
    etil2[:KS, :KS] = Et
    etil2[KS:, KS:] = Et
    etil2 = etil2.astype(bfloat16)

    e_start = np.exp(trans[:KR, START])
    w1 = g[:, 0, :].copy()
    w1[:, :KR] *= e_start[None, :]
    w1[:, HOLD] = 0.0

    # sorted strided assignment: core k column c -> batch order[c*8+k]
    order = np.argsort(seq_len, kind="stable")
    asg = order.reshape(BC, NCORES)            # [c, core]

    # chain-step -> step table per (pair, half): 0-indexed into T axis, T=dead
    tmap = np.full((PAIRS, 2, nstep), T, dtype=np.int64)
    for j in range(PAIRS):
        for half in range(2):
            seg = 2 * j + half
            if seg == 0:
                for i in range(C - 1):
                    tmap[j, half, i] = 1 + i
            else:
                for i in range(nstep):
                    if i < w:
                        t = seg * C - w + 1 + i
                    else:
                        t = seg * C + 1 + (i - w)
                    tmap[j, half, i] = t - 1

    probe = np.zeros(KS, dtype=np.float32)
    probe[:KR] = 1.0 / KR

    gpad = np.concatenate([g, np.zeros((B, 1, KS), np.float32)], axis=1)
    galls, winits = [], []
    for cix in range(NCORES):
        bidx = asg[:, cix]                     # [BC] batch ids, ascending L
        sub = gpad[bidx]                       # [BC, T+1, KS]
        ga = [np.zeros((P2, nstep, GWS[gi]), dtype=np.float32) for gi in range(NGRP)]
        wi = [np.zeros((P2, GWS[gi]), dtype=np.float32) for gi in range(NGRP)]
        for j in range(PAIRS):
            gi, off, wd, lo = j % NGRP, POFF[j], WID[j], LO[j]
            for half in range(2):
                rows = slice(half * KS, (half + 1) * KS)
                # emis [wd_cols, nstep, KS] -> [KS, nstep, wd]
                em = sub[lo:, tmap[j, half], :]
                ga[gi][rows, :, off : off + wd] = em.transpose(2, 1, 0)
            wi[gi][0:KS, off : off + wd] = probe[:, None]
            wi[gi][KS:, off : off + wd] = probe[:, None]
            if j == 0:
                wi[gi][0:KS, off : off + wd] = w1[bidx[lo:]].T
        galls.append([np.ascontiguousarray(a.astype(bfloat16)) for a in ga])
        winits.append([np.ascontiguousarray(a.astype(bfloat16)) for a in wi])

    return etil2, galls, winits, C_at_L, g, w1, asg


def _exact_logZ(feats, seq_len, trans, b):
    E64 = np.exp(trans.astype(np.float64)).T
    fv = np.full(K, NEG, dtype=np.float64)
    fv[START] = 0.0
    for t in range(int(seq_len[b])):
        m = fv.max()
        wv = np.exp(fv - m)
        fv = np.log(E64.T @ wv + 1e-300) + m + feats[b, t].astype(np.float64)
    m = fv.max()
    return float(
        np.log(np.exp(fv - m + trans[STOP, :].astype(np.float64)).sum()) + m
    )


def _gold_score(feats, tags, seq_len, trans):
    feats = np.asarray(feats, dtype=np.float32)
    tags = np.asarray(tags, dtype=np.int64)
    seq_len = np.asarray(seq_len, dtype=np.int64)
    trans = np.asarray(trans, dtype=np.float32)
    tags_ext = np.concatenate(
        [np.full((B, 1), START, dtype=np.int64), tags], axis=1
    )
    trans_sc = trans[tags_ext[:, 1:], tags_ext[:, :-1]]
    emit_sc = np.take_along_axis(feats, tags_ext[:, 1:, None], axis=2)[..., 0]
    mask = np.arange(T)[None, :] < seq_len[:, None]
    last_tag = np.take_along_axis(tags_ext, seq_len[:, None], axis=1)[:, 0]
    return (
        np.where(mask, trans_sc + emit_sc, 0.0).sum(1, dtype=np.float64)
        + trans[STOP, last_tag]
    )


def _combine(snaps_list, feats, seq_len, trans, C_at_L, w1, asg):
    """snaps_list: per-core list of [94, 3, GWS[g]] arrays per group."""
    seq_len = np.asarray(seq_len, dtype=np.int64)
    trans = np.asarray(trans, dtype=np.float32)
    qs = np.zeros((S, B, KS), dtype=np.float64)
    rs = np.zeros((S, B, KS), dtype=np.float64)
    for cix in range(NCORES):
        bidx = asg[:, cix]
        sn = [np.asarray(a).astype(np.float32) for a in snaps_list[cix]]
        for j in range(PAIRS):
            gi, off, wd, lo = j % NGRP, POFF[j], WID[j], LO[j]
            cols = bidx[lo:]
            for half in range(2):
                seg = 2 * j + half
                rows = slice(half * KS, (half + 1) * KS)
                qs[seg, cols] = sn[gi][rows, 0, off : off + wd].T
                rs[seg, cols] = sn[gi][rows, 1, off : off + wd].T
            if j == 0:
                rs[0, cols] = sn[gi][0:KS, 2, off : off + wd].T
    qs[0] = w1.astype(np.float64)

    def n1(v):
        return v[..., :KR].sum(axis=-1)

    lk = np.zeros(B)
    logk = np.zeros((S, B))
    with np.errstate(divide="ignore", invalid="ignore"):
        for s in range(1, S):
            lk = lk + np.log(n1(rs[s - 1])) - np.log(n1(qs[s]))
            logk[s] = lk
    s_cap = np.minimum(seq_len // C, S - 1)
    cap = rs[s_cap, np.arange(B), HOLD]
    full = seq_len == T
    fdot = (
        rs[S - 1][:, :KR] * np.exp(trans[STOP, :KR].astype(np.float64))[None, :]
    ).sum(axis=1)
    cap = np.where(full, fdot, cap)
    with np.errstate(divide="ignore", invalid="ignore"):
        logZ = np.log(cap) + logk[s_cap, np.arange(B)] + C_at_L

    # staircase violations: column c of core k excluded from pair j although
    # its capture chain needs it (L >= 2jC). Sorted columns make this a
    # boundary check; recompute those elements exactly on the host.
    feats32 = np.asarray(feats, dtype=np.float32)
    patched = 0
    for cix in range(NCORES):
        bidx = asg[:, cix]
        L = seq_len[bidx]
        for j in range(PAIRS):
            lo = LO[j]
            bad = np.nonzero(L[:lo] >= 2 * j * C)[0]
            for c in bad:
                b = int(bidx[c])
                logZ[b] = _exact_logZ(feats32, seq_len, trans, b)
                patched += 1
    bad = ~np.isfinite(logZ)
    for b in np.nonzero(bad)[0]:
        logZ[b] = _exact_logZ(feats32, seq_len, trans, b)
        patched += 1
    return logZ, patched


def kernel(feats, tags, seq_len, transitions):
    feats = np.asarray(feats)
    etil2, galls, winits, C_at_L, g, w1, asg = _host_prep(
        feats, seq_len, transitions
    )
    nc = _build_module()
    in_maps = []
    for cix in range(NCORES):
        m = {"etil": etil2}
        for gi in range(NGRP):
            m[f"gall{gi}"] = galls[cix][gi]
            m[f"winit{gi}"] = winits[cix][gi]
        in_maps.append(m)
    res = run_bass_kernel_spmd(nc, in_maps, list(range(NCORES)))
    snaps_list = [
        [res.results[cix][f"snaps{gi}"] for gi in range(NGRP)]
        for cix in range(NCORES)
    ]
    logZ, _ = _combine(snaps_list, feats, seq_len, transitions, C_at_L, w1, asg)
    gold = _gold_score(feats, tags, seq_len, transitions)
    return np.float32(np.mean(logZ - gold))


# revision 7
# speedup vs baseline: 1.5326x; 1.0521x over previous
"""CRF NLL loss on 8 Trainium2 NeuronCores - segmented-contraction forward algorithm
with a seq_len staircase.

Extends kernel2's segmented-contraction scheme: batch elements are sorted by
seq_len and dealt strided across cores, so each core's 64 columns (sorted
ascending by L) span the length distribution. Pair-chain j (segments 2j,2j+1)
only processes columns c >= lo_j = max(0, 4j - M): a column with L < 2jC has
its capture in an earlier segment, so later pairs can drop it. The static
staircase has margin M; the host verifies it against the actual lengths and
computes the rare violating elements exactly in f64 (a few ms each).

Device: 2 lockstep groups (even/odd pairs, widths ~344/316 instead of 512),
one fat bf16 matmul + one wide DVE multiply per group per chain step.
"""
import os
import numpy as np
from contextlib import ExitStack
from ml_dtypes import bfloat16

import concourse.bacc as bacc
import concourse.bass as bass
import concourse.tile as tile
from concourse import mybir
from concourse.bass_utils import run_bass_kernel_spmd

B, T, K = 512, 1024, 48
START, STOP = 46, 47
NEG = -10000.0
KR = 46
HOLD = 46
KS = 47
P2 = 2 * KS
NCORES = 8
BC = B // NCORES
C = 32
W = int(os.environ.get("K_W", "4"))
S = T // C          # 32
PAIRS = S // 2      # 16
NGRP = int(os.environ.get("K_NGRP", "3"))
MARGIN = int(os.environ.get("K_MARGIN", "0"))

# static staircase: pair j processes columns [LO[j], BC)
LO = [max(0, 4 * j - MARGIN) for j in range(PAIRS)]
WID = [BC - lo for lo in LO]
GPAIRS = [[j for j in range(PAIRS) if j % NGRP == g] for g in range(NGRP)]
GWS = [sum(WID[j] for j in gp) for gp in GPAIRS]   # group widths
# offset of pair j inside its group tile
POFF = {}
for g, gp in enumerate(GPAIRS):
    off = 0
    for j in gp:
        POFF[j] = off
        off += WID[j]

_nc_cache = {}


def _build_module(w=W, repeat=1):
    key = (w, repeat)
    if key in _nc_cache:
        return _nc_cache[key]
    nstep = w + C
    nc = bacc.Bacc(
        "TRN2",
        target_bir_lowering=False,
        debug=False,
        enable_asserts=False,
        num_devices=NCORES,
    )
    bf16 = mybir.dt.bfloat16
    f32 = mybir.dt.float32
    e_dram = nc.dram_tensor("etil", [P2, P2], bf16, kind="ExternalInput").ap()
    g_drams = [
        nc.dram_tensor(f"gall{g}", [P2, nstep, GWS[g]], bf16,
                       kind="ExternalInput").ap()
        for g in range(NGRP)
    ]
    w_drams = [
        nc.dram_tensor(f"winit{g}", [P2, GWS[g]], bf16, kind="ExternalInput").ap()
        for g in range(NGRP)
    ]
    snap_drams = [
        nc.dram_tensor(f"snaps{g}", [P2, 3, GWS[g]], bf16,
                       kind="ExternalOutput").ap()
        for g in range(NGRP)
    ]

    CH = 8
    chunks = []
    s0 = 0
    while s0 < nstep:
        chunks.append((s0, min(CH, nstep - s0)))
        s0 += CH

    with tile.TileContext(nc) as tc:
        with ExitStack() as ctx:
            const = ctx.enter_context(tc.tile_pool(name="const", bufs=1))
            wpool = ctx.enter_context(tc.tile_pool(name="wp", bufs=2))
            gpool = ctx.enter_context(tc.tile_pool(name="gp", bufs=1))
            psum_p = ctx.enter_context(tc.tile_pool(name="ps", bufs=2, space="PSUM"))

            etile = const.tile([P2, P2], bf16)
            nc.sync.dma_start(out=etile, in_=e_dram)

            wcur = []
            for g in range(NGRP):
                wt = wpool.tile([P2, GWS[g]], bf16, tag=f"w{g}")
                nc.sync.dma_start(out=wt, in_=w_drams[g])
                wcur.append(wt)

            gt = {}
            for ci, (c0, cl) in enumerate(chunks):
                for g in range(NGRP):
                    t = gpool.tile([P2, cl, GWS[g]], bf16, tag=f"g{g}c{ci}")
                    nc.sync.dma_start(out=t, in_=g_drams[g][:, c0 : c0 + cl, :])
                    gt[(g, ci)] = t

            for rep in range(repeat):
                for i in range(nstep):
                    ci, off = divmod(i, CH)
                    for g in range(NGRP):
                        ps = psum_p.tile([P2, GWS[g]], f32, tag=f"ps{g}")
                        nc.tensor.matmul(ps, etile, wcur[g], start=True, stop=True)
                        wn = wpool.tile([P2, GWS[g]], bf16, tag=f"w{g}")
                        nc.vector.tensor_mul(wn, ps, gt[(g, ci)][:, off, :])
                        wcur[g] = wn
                        if rep == 0:
                            if i == w - 1:
                                nc.sync.dma_start(out=snap_drams[g][:, 0, :], in_=wn)
                            if i == nstep - 1:
                                nc.sync.dma_start(out=snap_drams[g][:, 1, :], in_=wn)
                            if i == C - 2 and g == 0:
                                nc.sync.dma_start(out=snap_drams[g][:, 2, :], in_=wn)

    nc.compile()
    _nc_cache[key] = nc
    return nc


def _build_timing(reps, w=W, nochain=False):
    """Timing-only module: the 38-step pass wrapped in a hardware For_i loop.
    Results are garbage (state carries across iterations); per-pass time =
    slope between two reps values. nochain=True makes matmuls read the initial
    w tile so the PE->DVE->PE roundtrip is broken (pure throughput measure)."""
    key = ("timing", w, reps, nochain)
    if key in _nc_cache:
        return _nc_cache[key]
    nstep = w + C
    nc = bacc.Bacc(
        "TRN2",
        target_bir_lowering=False,
        debug=False,
        enable_asserts=False,
        num_devices=NCORES,
    )
    bf16 = mybir.dt.bfloat16
    f32 = mybir.dt.float32
    e_dram = nc.dram_tensor("etil", [P2, P2], bf16, kind="ExternalInput").ap()
    g_drams = [
        nc.dram_tensor(f"gall{g}", [P2, nstep, GWS[g]], bf16,
                       kind="ExternalInput").ap()
        for g in range(NGRP)
    ]
    w_drams = [
        nc.dram_tensor(f"winit{g}", [P2, GWS[g]], bf16, kind="ExternalInput").ap()
        for g in range(NGRP)
    ]
    snap_drams = [
        nc.dram_tensor(f"snaps{g}", [P2, 3, GWS[g]], bf16,
                       kind="ExternalOutput").ap()
        for g in range(NGRP)
    ]
    CH = 8
    chunks = []
    s0 = 0
    while s0 < nstep:
        chunks.append((s0, min(CH, nstep - s0)))
        s0 += CH

    with tile.TileContext(nc) as tc:
        with ExitStack() as ctx:
            const = ctx.enter_context(tc.tile_pool(name="const", bufs=1))
            wpool = ctx.enter_context(tc.tile_pool(name="wp", bufs=4))
            gpool = ctx.enter_context(tc.tile_pool(name="gp", bufs=1))
            psum_p = ctx.enter_context(tc.tile_pool(name="ps", bufs=2, space="PSUM"))

            etile = const.tile([P2, P2], bf16)
            nc.sync.dma_start(out=etile, in_=e_dram)
            wcur = []
            for g in range(NGRP):
                wt = wpool.tile([P2, GWS[g]], bf16, tag=f"w{g}")
                nc.sync.dma_start(out=wt, in_=w_drams[g])
                wcur.append(wt)
            gt = {}
            for ci, (c0, cl) in enumerate(chunks):
                for g in range(NGRP):
                    t = gpool.tile([P2, cl, GWS[g]], bf16, tag=f"g{g}c{ci}")
                    nc.sync.dma_start(out=t, in_=g_drams[g][:, c0 : c0 + cl, :])
                    gt[(g, ci)] = t

            winit_tiles = list(wcur)
            with tc.For_i(0, reps, 1):
                for i in range(nstep):
                    ci, off = divmod(i, CH)
                    for g in range(NGRP):
                        ps = psum_p.tile([P2, GWS[g]], f32, tag=f"ps{g}")
                        src = winit_tiles[g] if nochain else wcur[g]
                        nc.tensor.matmul(ps, etile, src, start=True, stop=True)
                        wn = wpool.tile([P2, GWS[g]], bf16, tag=f"w{g}")
                        nc.vector.tensor_mul(wn, ps, gt[(g, ci)][:, off, :])
                        wcur[g] = wn

            for g in range(NGRP):
                for slot in range(3):
                    nc.sync.dma_start(out=snap_drams[g][:, slot, :], in_=wcur[g])

    nc.compile()
    _nc_cache[key] = nc
    return nc


def _shifts_and_g(feats, seq_len, trans):
    mx = feats.max(axis=2)
    E64 = np.exp(trans.astype(np.float64)).T
    drift = []
    for b in range(6):
        fv = np.full(K, NEG, dtype=np.float64)
        fv[START] = 0.0
        for t in range(min(int(seq_len[b]), 256)):
            m = fv.max()
            wv = np.exp(fv - m)
            fv = np.log(E64.T @ wv + 1e-300) + m + feats[b, t]
            drift.append((fv.max() - m) - mx[b, t])
    mu = float(np.mean(drift))
    c = mx + mu
    Ccum = np.cumsum(c, axis=1, dtype=np.float64)
    C_at_L = Ccum[np.arange(B), seq_len - 1]

    taus = np.arange(1, T + 1)
    live = taus[None, :] <= seq_len[:, None]
    g = np.zeros((B, T, KS), dtype=np.float32)
    g[:, :, :KR] = np.exp(feats[:, :, :KR] - c[:, :, None]) * live[:, :, None]
    g[:, :, HOLD] = (taus[None, :] >= (seq_len[:, None] + 1)).astype(np.float32)
    return g, C_at_L


def _host_prep(feats, seq_len, trans, w=W):
    feats = np.ascontiguousarray(feats, dtype=np.float32)
    seq_len = np.asarray(seq_len, dtype=np.int64)
    trans = np.asarray(trans, dtype=np.float32)
    nstep = w + C

    g, C_at_L = _shifts_and_g(feats, seq_len, trans)

    Et = np.zeros((KS, KS), dtype=np.float32)
    Et[:KR, :KR] = np.exp(trans[:KR, :KR]).T
    Et[:KR, HOLD] = np.exp(trans[STOP, :KR])
    Et[HOLD, HOLD] = 1.0
    etil2 = np.zeros((P2, P2), dtype=np.float32)
    etil2[:KS, :KS] = Et
    etil2[KS:, KS:] = Et
    etil2 = etil2.astype(bfloat16)

    e_start = np.exp(trans[:KR, START])
    w1 = g[:, 0, :].copy()
    w1[:, :KR] *= e_start[None, :]
    w1[:, HOLD] = 0.0

    # sorted strided assignment: core k column c -> batch order[c*8+k]
    order = np.argsort(seq_len, kind="stable")
    asg = order.reshape(BC, NCORES)            # [c, core]

    # chain-step -> step table per (pair, half): 0-indexed into T axis, T=dead
    tmap = np.full((PAIRS, 2, nstep), T, dtype=np.int64)
    for j in range(PAIRS):
        for half in range(2):
            seg = 2 * j + half
            if seg == 0:
                for i in range(C - 1):
                    tmap[j, half, i] = 1 + i
            else:
                for i in range(nstep):
                    if i < w:
                        t = seg * C - w + 1 + i
                    else:
                        t = seg * C + 1 + (i - w)
                    tmap[j, half, i] = t - 1

    probe = np.zeros(KS, dtype=np.float32)
    probe[:KR] = 1.0 / KR

    gpad = np.concatenate([g, np.zeros((B, 1, KS), np.float32)], axis=1)
    galls, winits = [], []
    for cix in range(NCORES):
        bidx = asg[:, cix]                     # [BC] batch ids, ascending L
        sub = gpad[bidx]                       # [BC, T+1, KS]
        ga = [np.zeros((P2, nstep, GWS[gi]), dtype=np.float32) for gi in range(NGRP)]
        wi = [np.zeros((P2, GWS[gi]), dtype=np.float32) for gi in range(NGRP)]
        for j in range(PAIRS):
            gi, off, wd, lo = j % NGRP, POFF[j], WID[j], LO[j]
            for half in range(2):
                rows = slice(half * KS, (half + 1) * KS)
                # emis [wd_cols, nstep, KS] -> [KS, nstep, wd]
                em = sub[lo:, tmap[j, half], :]
                ga[gi][rows, :, off : off + wd] = em.transpose(2, 1, 0)
            wi[gi][0:KS, off : off + wd] = probe[:, None]
            wi[gi][KS:, off : off + wd] = probe[:, None]
            if j == 0:
                wi[gi][0:KS, off : off + wd] = w1[bidx[lo:]].T
        galls.append([np.ascontiguousarray(a.astype(bfloat16)) for a in ga])
        winits.append([np.ascontiguousarray(a.astype(bfloat16)) for a in wi])

    return etil2, galls, winits, C_at_L, g, w1, asg


def _exact_logZ(feats, seq_len, trans, b):
    E64 = np.exp(trans.astype(np.float64)).T
    fv = np.full(K, NEG, dtype=np.float64)
    fv[START] = 0.0
    for t in range(int(seq_len[b])):
        m = fv.max()
        wv = np.exp(fv - m)
        fv = np.log(E64.T @ wv + 1e-300) + m + feats[b, t].astype(np.float64)
    m = fv.max()
    return float(
        np.log(np.exp(fv - m + trans[STOP, :].astype(np.float64)).sum()) + m
    )


def _gold_score(feats, tags, seq_len, trans):
    feats = np.asarray(feats, dtype=np.float32)
    tags = np.asarray(tags, dtype=np.int64)
    seq_len = np.asarray(seq_len, dtype=np.int64)
    trans = np.asarray(trans, dtype=np.float32)
    tags_ext = np.concatenate(
        [np.full((B, 1), START, dtype=np.int64), tags], axis=1
    )
    trans_sc = trans[tags_ext[:, 1:], tags_ext[:, :-1]]
    emit_sc = np.take_along_axis(feats, tags_ext[:, 1:, None], axis=2)[..., 0]
    mask = np.arange(T)[None, :] < seq_len[:, None]
    last_tag = np.take_along_axis(tags_ext, seq_len[:, None], axis=1)[:, 0]
    return (
        np.where(mask, trans_sc + emit_sc, 0.0).sum(1, dtype=np.float64)
        + trans[STOP, last_tag]
    )


def _combine(snaps_list, feats, seq_len, trans, C_at_L, w1, asg):
    """snaps_list: per-core list of [94, 3, GWS[g]] arrays per group."""
    seq_len = np.asarray(seq_len, dtype=np.int64)
    trans = np.asarray(trans, dtype=np.float32)
    qs = np.zeros((S, B, KS), dtype=np.float64)
    rs = np.zeros((S, B, KS), dtype=np.float64)
    for cix in range(NCORES):
        bidx = asg[:, cix]
        sn = [np.asarray(a).astype(np.float32) for a in snaps_list[cix]]
        for j in range(PAIRS):
            gi, off, wd, lo = j % NGRP, POFF[j], WID[j], LO[j]
            cols = bidx[lo:]
            for half in range(2):
                seg = 2 * j + half
                rows = slice(half * KS, (half + 1) * KS)
                qs[seg, cols] = sn[gi][rows, 0, off : off + wd].T
                rs[seg, cols] = sn[gi][rows, 1, off : off + wd].T
            if j == 0:
                rs[0, cols] = sn[gi][0:KS, 2, off : off + wd].T
    qs[0] = w1.astype(np.float64)

    def n1(v):
        return v[..., :KR].sum(axis=-1)

    lk = np.zeros(B)
    logk = np.zeros((S, B))
    with np.errstate(divide="ignore", invalid="ignore"):
        for s in range(1, S):
            lk = lk + np.log(n1(rs[s - 1])) - np.log(n1(qs[s]))
            logk[s] = lk
    s_cap = np.minimum(seq_len // C, S - 1)
    cap = rs[s_cap, np.arange(B), HOLD]
    full = seq_len == T
    fdot = (
        rs[S - 1][:, :KR] * np.exp(trans[STOP, :KR].astype(np.float64))[None, :]
    ).sum(axis=1)
    cap = np.where(full, fdot, cap)
    with np.errstate(divide="ignore", invalid="ignore"):
        logZ = np.log(cap) + logk[s_cap, np.arange(B)] + C_at_L

    # staircase violations: column c of core k excluded from pair j although
    # its capture chain needs it (L >= 2jC). Sorted columns make this a
    # boundary check; recompute those elements exactly on the host.
    feats32 = np.asarray(feats, dtype=np.float32)
    patched = 0
    for cix in range(NCORES):
        bidx = asg[:, cix]
        L = seq_len[bidx]
        for j in range(PAIRS):
            lo = LO[j]
            bad = np.nonzero(L[:lo] >= 2 * j * C)[0]
            for c in bad:
                b = int(bidx[c])
                logZ[b] = _exact_logZ(feats32, seq_len, trans, b)
                patched += 1
    bad = ~np.isfinite(logZ)
    for b in np.nonzero(bad)[0]:
        logZ[b] = _exact_logZ(feats32, seq_len, trans, b)
        patched += 1
    return logZ, patched


def kernel(feats, tags, seq_len, transitions):
    feats = np.asarray(feats)
    etil2, galls, winits, C_at_L, g, w1, asg = _host_prep(
        feats, seq_len, transitions
    )
    nc = _build_module()
    in_maps = []
    for cix in range(NCORES):
        m = {"etil": etil2}
        for gi in range(NGRP):
            m[f"gall{gi}"] = galls[cix][gi]
            m[f"winit{gi}"] = winits[cix][gi]
        in_maps.append(m)
    res = run_bass_kernel_spmd(nc, in_maps, list(range(NCORES)))
    snaps_list = [
        [res.results[cix][f"snaps{gi}"] for gi in range(NGRP)]
        for cix in range(NCORES)
    ]
    logZ, _ = _combine(snaps_list, feats, seq_len, transitions, C_at_L, w1, asg)
    gold = _gold_score(feats, tags, seq_len, transitions)
    return np.float32(np.mean(logZ - gold))


# revision 10
# speedup vs baseline: 1.6009x; 1.0445x over previous
"""CRF NLL loss on 8 Trainium2 NeuronCores - segmented-contraction forward algorithm
with a seq_len staircase.

Extends kernel2's segmented-contraction scheme: batch elements are sorted by
seq_len and dealt strided across cores, so each core's 64 columns (sorted
ascending by L) span the length distribution. Pair-chain j (segments 2j,2j+1)
only processes columns c >= lo_j = max(0, 4j - M): a column with L < 2jC has
its capture in an earlier segment, so later pairs can drop it. The static
staircase has margin M; the host verifies it against the actual lengths and
computes the rare violating elements exactly in f64 (a few ms each).

Device: 2 lockstep groups (even/odd pairs, widths ~344/316 instead of 512),
one fat bf16 matmul + one wide DVE multiply per group per chain step.
"""
import os
import numpy as np
from contextlib import ExitStack
from ml_dtypes import bfloat16

import concourse.bacc as bacc
import concourse.bass as bass
import concourse.tile as tile
from concourse import mybir
from concourse.bass_utils import run_bass_kernel_spmd

B, T, K = 512, 1024, 48
START, STOP = 46, 47
NEG = -10000.0
KR = 46
HOLD = 46
KS = 47
P2 = 2 * KS
NCORES = 8
BC = B // NCORES
C = 32
W = int(os.environ.get("K_W", "3"))
S = T // C          # 32
PAIRS = S // 2      # 16
NGRP = int(os.environ.get("K_NGRP", "3"))
MARGIN = int(os.environ.get("K_MARGIN", "0"))

# static staircase: pair j processes columns [LO[j], BC)
LO = [max(0, 4 * j - MARGIN) for j in range(PAIRS)]
WID = [BC - lo for lo in LO]
# greedy-balanced group assignment (widest pair to lightest group)
GPAIRS = [[] for _ in range(NGRP)]
_gw = [0] * NGRP
for j in sorted(range(PAIRS), key=lambda j: -WID[j]):
    g = min(range(NGRP), key=lambda gi: _gw[gi])
    GPAIRS[g].append(j)
    _gw[g] += WID[j]
for gp in GPAIRS:
    gp.sort()
GWS = [sum(WID[j] for j in gp) for gp in GPAIRS]   # group widths
GRP_OF = {j: g for g, gp in enumerate(GPAIRS) for j in gp}
# offset of pair j inside its group tile
POFF = {}
for g, gp in enumerate(GPAIRS):
    off = 0
    for j in gp:
        POFF[j] = off
        off += WID[j]

_nc_cache = {}


def _build_module(w=W, repeat=1):
    key = (w, repeat)
    if key in _nc_cache:
        return _nc_cache[key]
    nstep = w + C
    nc = bacc.Bacc(
        "TRN2",
        target_bir_lowering=False,
        debug=False,
        enable_asserts=False,
        num_devices=NCORES,
    )
    bf16 = mybir.dt.bfloat16
    f32 = mybir.dt.float32
    e_dram = nc.dram_tensor("etil", [P2, P2], bf16, kind="ExternalInput").ap()
    g_drams = [
        nc.dram_tensor(f"gall{g}", [P2, nstep, GWS[g]], bf16,
                       kind="ExternalInput").ap()
        for g in range(NGRP)
    ]
    w_drams = [
        nc.dram_tensor(f"winit{g}", [P2, GWS[g]], bf16, kind="ExternalInput").ap()
        for g in range(NGRP)
    ]
    snap_drams = [
        nc.dram_tensor(f"snaps{g}", [P2, 3, GWS[g]], bf16,
                       kind="ExternalOutput").ap()
        for g in range(NGRP)
    ]

    CH = 8
    chunks = []
    s0 = 0
    while s0 < nstep:
        chunks.append((s0, min(CH, nstep - s0)))
        s0 += CH

    with tile.TileContext(nc) as tc:
        with ExitStack() as ctx:
            const = ctx.enter_context(tc.tile_pool(name="const", bufs=1))
            wpool = ctx.enter_context(tc.tile_pool(name="wp", bufs=4))
            gpool = ctx.enter_context(tc.tile_pool(name="gp", bufs=1))
            psum_p = ctx.enter_context(tc.tile_pool(name="ps", bufs=2, space="PSUM"))

            etile = const.tile([P2, P2], bf16)
            nc.sync.dma_start(out=etile, in_=e_dram)

            wcur = []
            for g in range(NGRP):
                wt = wpool.tile([P2, GWS[g]], bf16, tag=f"w{g}")
                nc.sync.dma_start(out=wt, in_=w_drams[g])
                wcur.append(wt)

            gt = {}
            for ci, (c0, cl) in enumerate(chunks):
                for g in range(NGRP):
                    t = gpool.tile([P2, cl, GWS[g]], bf16, tag=f"g{g}c{ci}")
                    nc.sync.dma_start(out=t, in_=g_drams[g][:, c0 : c0 + cl, :])
                    gt[(g, ci)] = t

            for rep in range(repeat):
                for i in range(nstep):
                    ci, off = divmod(i, CH)
                    for g in range(NGRP):
                        ps = psum_p.tile([P2, GWS[g]], f32, tag=f"ps{g}")
                        nc.tensor.matmul(ps, etile, wcur[g], start=True, stop=True)
                        wn = wpool.tile([P2, GWS[g]], bf16, tag=f"w{g}")
                        nc.vector.tensor_mul(wn, ps, gt[(g, ci)][:, off, :])
                        wcur[g] = wn
                        if rep == 0:
                            if i == w - 1:
                                nc.sync.dma_start(out=snap_drams[g][:, 0, :], in_=wn)
                            if i == nstep - 1:
                                nc.sync.dma_start(out=snap_drams[g][:, 1, :], in_=wn)
                            if i == C - 2 and g == GRP_OF[0]:
                                nc.sync.dma_start(out=snap_drams[g][:, 2, :], in_=wn)

    nc.compile()
    _nc_cache[key] = nc
    return nc


def _build_timing(reps, w=W, nochain=False):
    """Timing-only module: the 38-step pass wrapped in a hardware For_i loop.
    Results are garbage (state carries across iterations); per-pass time =
    slope between two reps values. nochain=True makes matmuls read the initial
    w tile so the PE->DVE->PE roundtrip is broken (pure throughput measure)."""
    key = ("timing", w, reps, nochain)
    if key in _nc_cache:
        return _nc_cache[key]
    nstep = w + C
    nc = bacc.Bacc(
        "TRN2",
        target_bir_lowering=False,
        debug=False,
        enable_asserts=False,
        num_devices=NCORES,
    )
    bf16 = mybir.dt.bfloat16
    f32 = mybir.dt.float32
    e_dram = nc.dram_tensor("etil", [P2, P2], bf16, kind="ExternalInput").ap()
    g_drams = [
        nc.dram_tensor(f"gall{g}", [P2, nstep, GWS[g]], bf16,
                       kind="ExternalInput").ap()
        for g in range(NGRP)
    ]
    w_drams = [
        nc.dram_tensor(f"winit{g}", [P2, GWS[g]], bf16, kind="ExternalInput").ap()
        for g in range(NGRP)
    ]
    snap_drams = [
        nc.dram_tensor(f"snaps{g}", [P2, 3, GWS[g]], bf16,
                       kind="ExternalOutput").ap()
        for g in range(NGRP)
    ]
    CH = 8
    chunks = []
    s0 = 0
    while s0 < nstep:
        chunks.append((s0, min(CH, nstep - s0)))
        s0 += CH

    with tile.TileContext(nc) as tc:
        with ExitStack() as ctx:
            const = ctx.enter_context(tc.tile_pool(name="const", bufs=1))
            wpool = ctx.enter_context(tc.tile_pool(name="wp", bufs=4))
            gpool = ctx.enter_context(tc.tile_pool(name="gp", bufs=1))
            psum_p = ctx.enter_context(tc.tile_pool(name="ps", bufs=2, space="PSUM"))

            etile = const.tile([P2, P2], bf16)
            nc.sync.dma_start(out=etile, in_=e_dram)
            wcur = []
            for g in range(NGRP):
                wt = wpool.tile([P2, GWS[g]], bf16, tag=f"w{g}")
                nc.sync.dma_start(out=wt, in_=w_drams[g])
                wcur.append(wt)
            gt = {}
            for ci, (c0, cl) in enumerate(chunks):
                for g in range(NGRP):
                    t = gpool.tile([P2, cl, GWS[g]], bf16, tag=f"g{g}c{ci}")
                    nc.sync.dma_start(out=t, in_=g_drams[g][:, c0 : c0 + cl, :])
                    gt[(g, ci)] = t

            winit_tiles = list(wcur)
            with tc.For_i(0, reps, 1):
                for i in range(nstep):
                    ci, off = divmod(i, CH)
                    for g in range(NGRP):
                        ps = psum_p.tile([P2, GWS[g]], f32, tag=f"ps{g}")
                        src = winit_tiles[g] if nochain else wcur[g]
                        nc.tensor.matmul(ps, etile, src, start=True, stop=True)
                        wn = wpool.tile([P2, GWS[g]], bf16, tag=f"w{g}")
                        nc.vector.tensor_mul(wn, ps, gt[(g, ci)][:, off, :])
                        wcur[g] = wn

            for g in range(NGRP):
                for slot in range(3):
                    nc.sync.dma_start(out=snap_drams[g][:, slot, :], in_=wcur[g])

    nc.compile()
    _nc_cache[key] = nc
    return nc


def _shifts_and_g(feats, seq_len, trans):
    mx = feats.max(axis=2)
    E64 = np.exp(trans.astype(np.float64)).T
    drift = []
    for b in range(6):
        fv = np.full(K, NEG, dtype=np.float64)
        fv[START] = 0.0
        for t in range(min(int(seq_len[b]), 256)):
            m = fv.max()
            wv = np.exp(fv - m)
            fv = np.log(E64.T @ wv + 1e-300) + m + feats[b, t]
            drift.append((fv.max() - m) - mx[b, t])
    mu = float(np.mean(drift))
    c = mx + mu
    Ccum = np.cumsum(c, axis=1, dtype=np.float64)
    C_at_L = Ccum[np.arange(B), seq_len - 1]

    taus = np.arange(1, T + 1)
    live = taus[None, :] <= seq_len[:, None]
    g = np.zeros((B, T, KS), dtype=np.float32)
    g[:, :, :KR] = np.exp(feats[:, :, :KR] - c[:, :, None]) * live[:, :, None]
    g[:, :, HOLD] = (taus[None, :] >= (seq_len[:, None] + 1)).astype(np.float32)
    return g, C_at_L


def _host_prep(feats, seq_len, trans, w=W):
    feats = np.ascontiguousarray(feats, dtype=np.float32)
    seq_len = np.asarray(seq_len, dtype=np.int64)
    trans = np.asarray(trans, dtype=np.float32)
    nstep = w + C

    g, C_at_L = _shifts_and_g(feats, seq_len, trans)

    Et = np.zeros((KS, KS), dtype=np.float32)
    Et[:KR, :KR] = np.exp(trans[:KR, :KR]).T
    Et[:KR, HOLD] = np.exp(trans[STOP, :KR])
    Et[HOLD, HOLD] = 1.0
    etil2 = np.zeros((P2, P2), dtype=np.float32)
    etil2[:KS, :KS] = Et
    etil2[KS:, KS:] = Et
    etil2 = etil2.astype(bfloat16)

    e_start = np.exp(trans[:KR, START])
    w1 = g[:, 0, :].copy()
    w1[:, :KR] *= e_start[None, :]
    w1[:, HOLD] = 0.0

    # sorted strided assignment: core k column c -> batch order[c*8+k]
    order = np.argsort(seq_len, kind="stable")
    asg = order.reshape(BC, NCORES)            # [c, core]

    # chain-step -> step table per (pair, half): 0-indexed into T axis, T=dead
    tmap = np.full((PAIRS, 2, nstep), T, dtype=np.int64)
    for j in range(PAIRS):
        for half in range(2):
            seg = 2 * j + half
            if seg == 0:
                for i in range(C - 1):
                    tmap[j, half, i] = 1 + i
            else:
                for i in range(nstep):
                    if i < w:
                        t = seg * C - w + 1 + i
                    else:
                        t = seg * C + 1 + (i - w)
                    tmap[j, half, i] = t - 1

    probe = np.zeros(KS, dtype=np.float32)
    probe[:KR] = 1.0 / KR

    gpad = np.concatenate([g, np.zeros((B, 1, KS), np.float32)], axis=1)
    galls, winits = [], []
    for cix in range(NCORES):
        bidx = asg[:, cix]                     # [BC] batch ids, ascending L
        sub = gpad[bidx]                       # [BC, T+1, KS]
        ga = [np.zeros((P2, nstep, GWS[gi]), dtype=np.float32) for gi in range(NGRP)]
        wi = [np.zeros((P2, GWS[gi]), dtype=np.float32) for gi in range(NGRP)]
        for j in range(PAIRS):
            gi, off, wd, lo = GRP_OF[j], POFF[j], WID[j], LO[j]
            for half in range(2):
                rows = slice(half * KS, (half + 1) * KS)
                # emis [wd_cols, nstep, KS] -> [KS, nstep, wd]
                em = sub[lo:, tmap[j, half], :]
                ga[gi][rows, :, off : off + wd] = em.transpose(2, 1, 0)
            wi[gi][0:KS, off : off + wd] = probe[:, None]
            wi[gi][KS:, off : off + wd] = probe[:, None]
            if j == 0:
                wi[gi][0:KS, off : off + wd] = w1[bidx[lo:]].T
        galls.append([np.ascontiguousarray(a.astype(bfloat16)) for a in ga])
        winits.append([np.ascontiguousarray(a.astype(bfloat16)) for a in wi])

    return etil2, galls, winits, C_at_L, g, w1, asg


def _exact_logZ(feats, seq_len, trans, b):
    E64 = np.exp(trans.astype(np.float64)).T
    fv = np.full(K, NEG, dtype=np.float64)
    fv[START] = 0.0
    for t in range(int(seq_len[b])):
        m = fv.max()
        wv = np.exp(fv - m)
        fv = np.log(E64.T @ wv + 1e-300) + m + feats[b, t].astype(np.float64)
    m = fv.max()
    return float(
        np.log(np.exp(fv - m + trans[STOP, :].astype(np.float64)).sum()) + m
    )


def _gold_score(feats, tags, seq_len, trans):
    feats = np.asarray(feats, dtype=np.float32)
    tags = np.asarray(tags, dtype=np.int64)
    seq_len = np.asarray(seq_len, dtype=np.int64)
    trans = np.asarray(trans, dtype=np.float32)
    tags_ext = np.concatenate(
        [np.full((B, 1), START, dtype=np.int64), tags], axis=1
    )
    trans_sc = trans[tags_ext[:, 1:], tags_ext[:, :-1]]
    emit_sc = np.take_along_axis(feats, tags_ext[:, 1:, None], axis=2)[..., 0]
    mask = np.arange(T)[None, :] < seq_len[:, None]
    last_tag = np.take_along_axis(tags_ext, seq_len[:, None], axis=1)[:, 0]
    return (
        np.where(mask, trans_sc + emit_sc, 0.0).sum(1, dtype=np.float64)
        + trans[STOP, last_tag]
    )


def _combine(snaps_list, feats, seq_len, trans, C_at_L, w1, asg):
    """snaps_list: per-core list of [94, 3, GWS[g]] arrays per group."""
    seq_len = np.asarray(seq_len, dtype=np.int64)
    trans = np.asarray(trans, dtype=np.float32)
    qs = np.zeros((S, B, KS), dtype=np.float64)
    rs = np.zeros((S, B, KS), dtype=np.float64)
    for cix in range(NCORES):
        bidx = asg[:, cix]
        sn = [np.asarray(a).astype(np.float32) for a in snaps_list[cix]]
        for j in range(PAIRS):
            gi, off, wd, lo = GRP_OF[j], POFF[j], WID[j], LO[j]
            cols = bidx[lo:]
            for half in range(2):
                seg = 2 * j + half
                rows = slice(half * KS, (half + 1) * KS)
                qs[seg, cols] = sn[gi][rows, 0, off : off + wd].T
                rs[seg, cols] = sn[gi][rows, 1, off : off + wd].T
            if j == 0:
                rs[0, cols] = sn[gi][0:KS, 2, off : off + wd].T
    qs[0] = w1.astype(np.float64)

    def n1(v):
        return v[..., :KR].sum(axis=-1)

    lk = np.zeros(B)
    logk = np.zeros((S, B))
    with np.errstate(divide="ignore", invalid="ignore"):
        for s in range(1, S):
            lk = lk + np.log(n1(rs[s - 1])) - np.log(n1(qs[s]))
            logk[s] = lk
    s_cap = np.minimum(seq_len // C, S - 1)
    cap = rs[s_cap, np.arange(B), HOLD]
    full = seq_len == T
    fdot = (
        rs[S - 1][:, :KR] * np.exp(trans[STOP, :KR].astype(np.float64))[None, :]
    ).sum(axis=1)
    cap = np.where(full, fdot, cap)
    with np.errstate(divide="ignore", invalid="ignore"):
        logZ = np.log(cap) + logk[s_cap, np.arange(B)] + C_at_L

    # staircase violations: column c of core k excluded from pair j although
    # its capture chain needs it (L >= 2jC). Sorted columns make this a
    # boundary check; recompute those elements exactly on the host.
    feats32 = np.asarray(feats, dtype=np.float32)
    patched = 0
    for cix in range(NCORES):
        bidx = asg[:, cix]
        L = seq_len[bidx]
        for j in range(PAIRS):
            lo = LO[j]
            bad = np.nonzero(L[:lo] >= 2 * j * C)[0]
            for c in bad:
                b = int(bidx[c])
                logZ[b] = _exact_logZ(feats32, seq_len, trans, b)
                patched += 1
    bad = ~np.isfinite(logZ)
    for b in np.nonzero(bad)[0]:
        logZ[b] = _exact_logZ(feats32, seq_len, trans, b)
        patched += 1
    return logZ, patched


def kernel(feats, tags, seq_len, transitions):
    feats = np.asarray(feats)
    etil2, galls, winits, C_at_L, g, w1, asg = _host_prep(
        feats, seq_len, transitions
    )
    nc = _build_module()
    in_maps = []
    for cix in range(NCORES):
        m = {"etil": etil2}
        for gi in range(NGRP):
            m[f"gall{gi}"] = galls[cix][gi]
            m[f"winit{gi}"] = winits[cix][gi]
        in_maps.append(m)
    res = run_bass_kernel_spmd(nc, in_maps, list(range(NCORES)))
    snaps_list = [
        [res.results[cix][f"snaps{gi}"] for gi in range(NGRP)]
        for cix in range(NCORES)
    ]
    logZ, _ = _combine(snaps_list, feats, seq_len, transitions, C_at_L, w1, asg)
    gold = _gold_score(feats, tags, seq_len, transitions)
    return np.float32(np.mean(logZ - gold))


# revision 11
# speedup vs baseline: 1.6579x; 1.0356x over previous
"""CRF NLL loss on 8 Trainium2 NeuronCores - segmented-contraction forward algorithm
with a seq_len staircase.

Extends kernel2's segmented-contraction scheme: batch elements are sorted by
seq_len and dealt strided across cores, so each core's 64 columns (sorted
ascending by L) span the length distribution. Pair-chain j (segments 2j,2j+1)
only processes columns c >= lo_j = max(0, 4j - M): a column with L < 2jC has
its capture in an earlier segment, so later pairs can drop it. The static
staircase has margin M; the host verifies it against the actual lengths and
computes the rare violating elements exactly in f64 (a few ms each).

Device: 2 lockstep groups (even/odd pairs, widths ~344/316 instead of 512),
one fat bf16 matmul + one wide DVE multiply per group per chain step.
"""
import os
import numpy as np
from contextlib import ExitStack
from ml_dtypes import bfloat16

import concourse.bacc as bacc
import concourse.bass as bass
import concourse.tile as tile
from concourse import mybir
from concourse.bass_utils import run_bass_kernel_spmd

B, T, K = 512, 1024, 48
START, STOP = 46, 47
NEG = -10000.0
KR = 46
HOLD = 46
KS = 47
P2 = 2 * KS
NCORES = 8
BC = B // NCORES
C = int(os.environ.get("K_C", "32"))
W = int(os.environ.get("K_W", "1"))
S = T // C          # 32
PAIRS = S // 2      # 16
NGRP = int(os.environ.get("K_NGRP", "3"))
MARGIN = int(os.environ.get("K_MARGIN", "0"))

# static staircase: pair j processes columns [LO[j], BC)
LO = [max(0, (C * j) // 8 - MARGIN) for j in range(PAIRS)]
WID = [BC - lo for lo in LO]
# greedy-balanced group assignment (widest pair to lightest group)
GPAIRS = [[] for _ in range(NGRP)]
_gw = [0] * NGRP
for j in sorted(range(PAIRS), key=lambda j: -WID[j]):
    g = min(range(NGRP), key=lambda gi: _gw[gi])
    GPAIRS[g].append(j)
    _gw[g] += WID[j]
for gp in GPAIRS:
    gp.sort()
GWS = [sum(WID[j] for j in gp) for gp in GPAIRS]   # group widths
GRP_OF = {j: g for g, gp in enumerate(GPAIRS) for j in gp}
# offset of pair j inside its group tile
POFF = {}
for g, gp in enumerate(GPAIRS):
    off = 0
    for j in gp:
        POFF[j] = off
        off += WID[j]

_nc_cache = {}


def _build_module(w=W, repeat=1):
    key = (w, repeat)
    if key in _nc_cache:
        return _nc_cache[key]
    nstep = w + C
    nc = bacc.Bacc(
        "TRN2",
        target_bir_lowering=False,
        debug=False,
        enable_asserts=False,
        num_devices=NCORES,
    )
    bf16 = mybir.dt.bfloat16
    f32 = mybir.dt.float32
    e_dram = nc.dram_tensor("etil", [P2, P2], bf16, kind="ExternalInput").ap()
    g_drams = [
        nc.dram_tensor(f"gall{g}", [P2, nstep, GWS[g]], bf16,
                       kind="ExternalInput").ap()
        for g in range(NGRP)
    ]
    w_drams = [
        nc.dram_tensor(f"winit{g}", [P2, GWS[g]], bf16, kind="ExternalInput").ap()
        for g in range(NGRP)
    ]
    snap_drams = [
        nc.dram_tensor(f"snaps{g}", [P2, 3, GWS[g]], bf16,
                       kind="ExternalOutput").ap()
        for g in range(NGRP)
    ]

    CH = 8
    chunks = []
    s0 = 0
    while s0 < nstep:
        chunks.append((s0, min(CH, nstep - s0)))
        s0 += CH

    with tile.TileContext(nc) as tc:
        with ExitStack() as ctx:
            const = ctx.enter_context(tc.tile_pool(name="const", bufs=1))
            wpool = ctx.enter_context(tc.tile_pool(name="wp", bufs=4))
            gpool = ctx.enter_context(tc.tile_pool(name="gp", bufs=1))
            psum_p = ctx.enter_context(tc.tile_pool(name="ps", bufs=2, space="PSUM"))

            etile = const.tile([P2, P2], bf16)
            nc.sync.dma_start(out=etile, in_=e_dram)

            wcur = []
            for g in range(NGRP):
                wt = wpool.tile([P2, GWS[g]], bf16, tag=f"w{g}")
                nc.sync.dma_start(out=wt, in_=w_drams[g])
                wcur.append(wt)

            gt = {}
            for ci, (c0, cl) in enumerate(chunks):
                for g in range(NGRP):
                    t = gpool.tile([P2, cl, GWS[g]], bf16, tag=f"g{g}c{ci}")
                    nc.sync.dma_start(out=t, in_=g_drams[g][:, c0 : c0 + cl, :])
                    gt[(g, ci)] = t

            for rep in range(repeat):
                for i in range(nstep):
                    ci, off = divmod(i, CH)
                    for g in range(NGRP):
                        ps = psum_p.tile([P2, GWS[g]], f32, tag=f"ps{g}")
                        nc.tensor.matmul(ps, etile, wcur[g], start=True, stop=True)
                        wn = wpool.tile([P2, GWS[g]], bf16, tag=f"w{g}")
                        nc.vector.tensor_mul(wn, ps, gt[(g, ci)][:, off, :])
                        wcur[g] = wn
                        if rep == 0:
                            if i == w - 1:
                                nc.sync.dma_start(out=snap_drams[g][:, 0, :], in_=wn)
                            if i == nstep - 1:
                                nc.sync.dma_start(out=snap_drams[g][:, 1, :], in_=wn)
                            if i == C - 2 and g == GRP_OF[0]:
                                nc.sync.dma_start(out=snap_drams[g][:, 2, :], in_=wn)

    nc.compile()
    _nc_cache[key] = nc
    return nc


def _build_timing(reps, w=W, nochain=False):
    """Timing-only module: the 38-step pass wrapped in a hardware For_i loop.
    Results are garbage (state carries across iterations); per-pass time =
    slope between two reps values. nochain=True makes matmuls read the initial
    w tile so the PE->DVE->PE roundtrip is broken (pure throughput measure)."""
    key = ("timing", w, reps, nochain)
    if key in _nc_cache:
        return _nc_cache[key]
    nstep = w + C
    nc = bacc.Bacc(
        "TRN2",
        target_bir_lowering=False,
        debug=False,
        enable_asserts=False,
        num_devices=NCORES,
    )
    bf16 = mybir.dt.bfloat16
    f32 = mybir.dt.float32
    e_dram = nc.dram_tensor("etil", [P2, P2], bf16, kind="ExternalInput").ap()
    g_drams = [
        nc.dram_tensor(f"gall{g}", [P2, nstep, GWS[g]], bf16,
                       kind="ExternalInput").ap()
        for g in range(NGRP)
    ]
    w_drams = [
        nc.dram_tensor(f"winit{g}", [P2, GWS[g]], bf16, kind="ExternalInput").ap()
        for g in range(NGRP)
    ]
    snap_drams = [
        nc.dram_tensor(f"snaps{g}", [P2, 3, GWS[g]], bf16,
                       kind="ExternalOutput").ap()
        for g in range(NGRP)
    ]
    CH = 8
    chunks = []
    s0 = 0
    while s0 < nstep:
        chunks.append((s0, min(CH, nstep - s0)))
        s0 += CH

    with tile.TileContext(nc) as tc:
        with ExitStack() as ctx:
            const = ctx.enter_context(tc.tile_pool(name="const", bufs=1))
            wpool = ctx.enter_context(tc.tile_pool(name="wp", bufs=4))
            gpool = ctx.enter_context(tc.tile_pool(name="gp", bufs=1))
            psum_p = ctx.enter_context(tc.tile_pool(name="ps", bufs=2, space="PSUM"))

            etile = const.tile([P2, P2], bf16)
            nc.sync.dma_start(out=etile, in_=e_dram)
            wcur = []
            for g in range(NGRP):
                wt = wpool.tile([P2, GWS[g]], bf16, tag=f"w{g}")
                nc.sync.dma_start(out=wt, in_=w_drams[g])
                wcur.append(wt)
            gt = {}
            for ci, (c0, cl) in enumerate(chunks):
                for g in range(NGRP):
                    t = gpool.tile([P2, cl, GWS[g]], bf16, tag=f"g{g}c{ci}")
                    nc.sync.dma_start(out=t, in_=g_drams[g][:, c0 : c0 + cl, :])
                    gt[(g, ci)] = t

            winit_tiles = list(wcur)
            with tc.For_i(0, reps, 1):
                for i in range(nstep):
                    ci, off = divmod(i, CH)
                    for g in range(NGRP):
                        ps = psum_p.tile([P2, GWS[g]], f32, tag=f"ps{g}")
                        src = winit_tiles[g] if nochain else wcur[g]
                        nc.tensor.matmul(ps, etile, src, start=True, stop=True)
                        wn = wpool.tile([P2, GWS[g]], bf16, tag=f"w{g}")
                        nc.vector.tensor_mul(wn, ps, gt[(g, ci)][:, off, :])
                        wcur[g] = wn

            for g in range(NGRP):
                for slot in range(3):
                    nc.sync.dma_start(out=snap_drams[g][:, slot, :], in_=wcur[g])

    nc.compile()
    _nc_cache[key] = nc
    return nc


def _shifts_and_g(feats, seq_len, trans):
    mx = feats.max(axis=2)
    E64 = np.exp(trans.astype(np.float64)).T
    drift = []
    for b in range(6):
        fv = np.full(K, NEG, dtype=np.float64)
        fv[START] = 0.0
        for t in range(min(int(seq_len[b]), 256)):
            m = fv.max()
            wv = np.exp(fv - m)
            fv = np.log(E64.T @ wv + 1e-300) + m + feats[b, t]
            drift.append((fv.max() - m) - mx[b, t])
    mu = float(np.mean(drift))
    c = mx + mu
    Ccum = np.cumsum(c, axis=1, dtype=np.float64)
    C_at_L = Ccum[np.arange(B), seq_len - 1]

    taus = np.arange(1, T + 1)
    live = taus[None, :] <= seq_len[:, None]
    g = np.zeros((B, T, KS), dtype=np.float32)
    g[:, :, :KR] = np.exp(feats[:, :, :KR] - c[:, :, None]) * live[:, :, None]
    g[:, :, HOLD] = (taus[None, :] >= (seq_len[:, None] + 1)).astype(np.float32)
    return g, C_at_L


def _host_prep(feats, seq_len, trans, w=W):
    feats = np.ascontiguousarray(feats, dtype=np.float32)
    seq_len = np.asarray(seq_len, dtype=np.int64)
    trans = np.asarray(trans, dtype=np.float32)
    nstep = w + C

    g, C_at_L = _shifts_and_g(feats, seq_len, trans)

    Et = np.zeros((KS, KS), dtype=np.float32)
    Et[:KR, :KR] = np.exp(trans[:KR, :KR]).T
    Et[:KR, HOLD] = np.exp(trans[STOP, :KR])
    Et[HOLD, HOLD] = 1.0
    etil2 = np.zeros((P2, P2), dtype=np.float32)
    etil2[:KS, :KS] = Et
    etil2[KS:, KS:] = Et
    etil2 = etil2.astype(bfloat16)

    e_start = np.exp(trans[:KR, START])
    w1 = g[:, 0, :].copy()
    w1[:, :KR] *= e_start[None, :]
    w1[:, HOLD] = 0.0

    # sorted strided assignment: core k column c -> batch order[c*8+k]
    order = np.argsort(seq_len, kind="stable")
    asg = order.reshape(BC, NCORES)            # [c, core]

    # chain-step -> step table per (pair, half): 0-indexed into T axis, T=dead
    tmap = np.full((PAIRS, 2, nstep), T, dtype=np.int64)
    for j in range(PAIRS):
        for half in range(2):
            seg = 2 * j + half
            if seg == 0:
                for i in range(C - 1):
                    tmap[j, half, i] = 1 + i
            else:
                for i in range(nstep):
                    if i < w:
                        t = seg * C - w + 1 + i
                    else:
                        t = seg * C + 1 + (i - w)
                    tmap[j, half, i] = t - 1

    probe = np.zeros(KS, dtype=np.float32)
    probe[:KR] = 1.0 / KR

    gpad = np.concatenate([g, np.zeros((B, 1, KS), np.float32)], axis=1)
    galls, winits = [], []
    for cix in range(NCORES):
        bidx = asg[:, cix]                     # [BC] batch ids, ascending L
        sub = gpad[bidx]                       # [BC, T+1, KS]
        ga = [np.zeros((P2, nstep, GWS[gi]), dtype=np.float32) for gi in range(NGRP)]
        wi = [np.zeros((P2, GWS[gi]), dtype=np.float32) for gi in range(NGRP)]
        for j in range(PAIRS):
            gi, off, wd, lo = GRP_OF[j], POFF[j], WID[j], LO[j]
            for half in range(2):
                rows = slice(half * KS, (half + 1) * KS)
                # emis [wd_cols, nstep, KS] -> [KS, nstep, wd]
                em = sub[lo:, tmap[j, half], :]
                ga[gi][rows, :, off : off + wd] = em.transpose(2, 1, 0)
            wi[gi][0:KS, off : off + wd] = probe[:, None]
            wi[gi][KS:, off : off + wd] = probe[:, None]
            if j == 0:
                wi[gi][0:KS, off : off + wd] = w1[bidx[lo:]].T
        galls.append([np.ascontiguousarray(a.astype(bfloat16)) for a in ga])
        winits.append([np.ascontiguousarray(a.astype(bfloat16)) for a in wi])

    return etil2, galls, winits, C_at_L, g, w1, asg


def _exact_logZ(feats, seq_len, trans, b):
    E64 = np.exp(trans.astype(np.float64)).T
    fv = np.full(K, NEG, dtype=np.float64)
    fv[START] = 0.0
    for t in range(int(seq_len[b])):
        m = fv.max()
        wv = np.exp(fv - m)
        fv = np.log(E64.T @ wv + 1e-300) + m + feats[b, t].astype(np.float64)
    m = fv.max()
    return float(
        np.log(np.exp(fv - m + trans[STOP, :].astype(np.float64)).sum()) + m
    )


def _gold_score(feats, tags, seq_len, trans):
    feats = np.asarray(feats, dtype=np.float32)
    tags = np.asarray(tags, dtype=np.int64)
    seq_len = np.asarray(seq_len, dtype=np.int64)
    trans = np.asarray(trans, dtype=np.float32)
    tags_ext = np.concatenate(
        [np.full((B, 1), START, dtype=np.int64), tags], axis=1
    )
    trans_sc = trans[tags_ext[:, 1:], tags_ext[:, :-1]]
    emit_sc = np.take_along_axis(feats, tags_ext[:, 1:, None], axis=2)[..., 0]
    mask = np.arange(T)[None, :] < seq_len[:, None]
    last_tag = np.take_along_axis(tags_ext, seq_len[:, None], axis=1)[:, 0]
    return (
        np.where(mask, trans_sc + emit_sc, 0.0).sum(1, dtype=np.float64)
        + trans[STOP, last_tag]
    )


def _combine(snaps_list, feats, seq_len, trans, C_at_L, w1, asg):
    """snaps_list: per-core list of [94, 3, GWS[g]] arrays per group."""
    seq_len = np.asarray(seq_len, dtype=np.int64)
    trans = np.asarray(trans, dtype=np.float32)
    qs = np.zeros((S, B, KS), dtype=np.float64)
    rs = np.zeros((S, B, KS), dtype=np.float64)
    for cix in range(NCORES):
        bidx = asg[:, cix]
        sn = [np.asarray(a).astype(np.float32) for a in snaps_list[cix]]
        for j in range(PAIRS):
            gi, off, wd, lo = GRP_OF[j], POFF[j], WID[j], LO[j]
            cols = bidx[lo:]
            for half in range(2):
                seg = 2 * j + half
                rows = slice(half * KS, (half + 1) * KS)
                qs[seg, cols] = sn[gi][rows, 0, off : off + wd].T
                rs[seg, cols] = sn[gi][rows, 1, off : off + wd].T
            if j == 0:
                rs[0, cols] = sn[gi][0:KS, 2, off : off + wd].T
    qs[0] = w1.astype(np.float64)

    def n1(v):
        return v[..., :KR].sum(axis=-1)

    lk = np.zeros(B)
    logk = np.zeros((S, B))
    with np.errstate(divide="ignore", invalid="ignore"):
        for s in range(1, S):
            lk = lk + np.log(n1(rs[s - 1])) - np.log(n1(qs[s]))
            logk[s] = lk
    s_cap = np.minimum(seq_len // C, S - 1)
    cap = rs[s_cap, np.arange(B), HOLD]
    full = seq_len == T
    fdot = (
        rs[S - 1][:, :KR] * np.exp(trans[STOP, :KR].astype(np.float64))[None, :]
    ).sum(axis=1)
    cap = np.where(full, fdot, cap)
    with np.errstate(divide="ignore", invalid="ignore"):
        logZ = np.log(cap) + logk[s_cap, np.arange(B)] + C_at_L

    # staircase violations: column c of core k excluded from pair j although
    # its capture chain needs it (L >= 2jC). Sorted columns make this a
    # boundary check; recompute those elements exactly on the host.
    feats32 = np.asarray(feats, dtype=np.float32)
    patched = 0
    for cix in range(NCORES):
        bidx = asg[:, cix]
        L = seq_len[bidx]
        for j in range(PAIRS):
            lo = LO[j]
            bad = np.nonzero(L[:lo] >= 2 * j * C)[0]
            for c in bad:
                b = int(bidx[c])
                logZ[b] = _exact_logZ(feats32, seq_len, trans, b)
                patched += 1
    bad = ~np.isfinite(logZ)
    for b in np.nonzero(bad)[0]:
        logZ[b] = _exact_logZ(feats32, seq_len, trans, b)
        patched += 1
    return logZ, patched


def kernel(feats, tags, seq_len, transitions):
    feats = np.asarray(feats)
    etil2, galls, winits, C_at_L, g, w1, asg = _host_prep(
        feats, seq_len, transitions
    )
    nc = _build_module()
    in_maps = []
    for cix in range(NCORES):
        m = {"etil": etil2}
        for gi in range(NGRP):
            m[f"gall{gi}"] = galls[cix][gi]
            m[f"winit{gi}"] = winits[cix][gi]
        in_maps.append(m)
    res = run_bass_kernel_spmd(nc, in_maps, list(range(NCORES)))
    snaps_list = [
        [res.results[cix][f"snaps{gi}"] for gi in range(NGRP)]
        for cix in range(NCORES)
    ]
    logZ, _ = _combine(snaps_list, feats, seq_len, transitions, C_at_L, w1, asg)
    gold = _gold_score(feats, tags, seq_len, transitions)
    return np.float32(np.mean(logZ - gold))


# revision 12
# speedup vs baseline: 1.7638x; 1.0639x over previous
"""CRF NLL loss on 8 Trainium2 NeuronCores - segmented-contraction forward algorithm
with a seq_len staircase.

Extends kernel2's segmented-contraction scheme: batch elements are sorted by
seq_len and dealt strided across cores, so each core's 64 columns (sorted
ascending by L) span the length distribution. Pair-chain j (segments 2j,2j+1)
only processes columns c >= lo_j = max(0, 4j - M): a column with L < 2jC has
its capture in an earlier segment, so later pairs can drop it. The static
staircase has margin M; the host verifies it against the actual lengths and
computes the rare violating elements exactly in f64 (a few ms each).

Device: 2 lockstep groups (even/odd pairs, widths ~344/316 instead of 512),
one fat bf16 matmul + one wide DVE multiply per group per chain step.
"""
import os
import numpy as np
from contextlib import ExitStack
from ml_dtypes import bfloat16

import concourse.bacc as bacc
import concourse.bass as bass
import concourse.tile as tile
from concourse import mybir
from concourse.bass_utils import run_bass_kernel_spmd

B, T, K = 512, 1024, 48
START, STOP = 46, 47
NEG = -10000.0
KR = 46
HOLD = 46
KS = 47
P2 = 2 * KS
NCORES = 8
BC = B // NCORES
C = int(os.environ.get("K_C", "32"))
W = int(os.environ.get("K_W", "1"))
S = T // C          # 32
PAIRS = S // 2      # 16
NGRP = int(os.environ.get("K_NGRP", "3"))
MARGIN = int(os.environ.get("K_MARGIN", "0"))

# static staircase: pair j processes columns [LO[j], BC)
LO = [max(0, (C * j) // 8 - MARGIN) for j in range(PAIRS)]
WID = [BC - lo for lo in LO]
# greedy-balanced group assignment (widest pair to lightest group)
GPAIRS = [[] for _ in range(NGRP)]
_gw = [0] * NGRP
for j in sorted(range(PAIRS), key=lambda j: -WID[j]):
    g = min(range(NGRP), key=lambda gi: _gw[gi])
    GPAIRS[g].append(j)
    _gw[g] += WID[j]
for gp in GPAIRS:
    gp.sort()
GWS = [sum(WID[j] for j in gp) for gp in GPAIRS]   # group widths
GRP_OF = {j: g for g, gp in enumerate(GPAIRS) for j in gp}
# offset of pair j inside its group tile
POFF = {}
for g, gp in enumerate(GPAIRS):
    off = 0
    for j in gp:
        POFF[j] = off
        off += WID[j]

_nc_cache = {}


def _build_module(w=W, repeat=1):
    key = (w, repeat)
    if key in _nc_cache:
        return _nc_cache[key]
    nstep = w + C
    nc = bacc.Bacc(
        "TRN2",
        target_bir_lowering=False,
        debug=False,
        enable_asserts=False,
        num_devices=NCORES,
    )
    bf16 = mybir.dt.bfloat16
    f32 = mybir.dt.float32
    e_dram = nc.dram_tensor("etil", [P2, P2], bf16, kind="ExternalInput").ap()
    g_drams = [
        nc.dram_tensor(f"gall{g}", [P2, nstep, GWS[g]], bf16,
                       kind="ExternalInput").ap()
        for g in range(NGRP)
    ]
    w_drams = [
        nc.dram_tensor(f"winit{g}", [P2, GWS[g]], bf16, kind="ExternalInput").ap()
        for g in range(NGRP)
    ]
    snap_drams = [
        nc.dram_tensor(f"snaps{g}", [P2, 3, GWS[g]], bf16,
                       kind="ExternalOutput").ap()
        for g in range(NGRP)
    ]

    chunks = []
    s0 = 0
    for cl in (2, 3, 4):          # small leading chunks: compute starts early
        if s0 < nstep:
            chunks.append((s0, min(cl, nstep - s0)))
            s0 += cl
    while s0 < nstep:
        chunks.append((s0, min(8, nstep - s0)))
        s0 += 8
    step_chunk = {}
    for ci, (c0, cl) in enumerate(chunks):
        for i in range(c0, c0 + cl):
            step_chunk[i] = (ci, i - c0)

    with tile.TileContext(nc) as tc:
        with ExitStack() as ctx:
            const = ctx.enter_context(tc.tile_pool(name="const", bufs=1))
            wpool = ctx.enter_context(tc.tile_pool(name="wp", bufs=4))
            gpool = ctx.enter_context(tc.tile_pool(name="gp", bufs=1))
            psum_p = ctx.enter_context(tc.tile_pool(name="ps", bufs=2, space="PSUM"))

            etile = const.tile([P2, P2], bf16)
            nc.sync.dma_start(out=etile, in_=e_dram)

            wcur = []
            for g in range(NGRP):
                wt = wpool.tile([P2, GWS[g]], bf16, tag=f"w{g}")
                nc.sync.dma_start(out=wt, in_=w_drams[g])
                wcur.append(wt)

            gt = {}
            for ci, (c0, cl) in enumerate(chunks):
                for g in range(NGRP):
                    t = gpool.tile([P2, cl, GWS[g]], bf16, tag=f"g{g}c{ci}")
                    nc.sync.dma_start(out=t, in_=g_drams[g][:, c0 : c0 + cl, :])
                    gt[(g, ci)] = t

            for rep in range(repeat):
                for i in range(nstep):
                    ci, off = step_chunk[i]
                    for g in range(NGRP):
                        ps = psum_p.tile([P2, GWS[g]], f32, tag=f"ps{g}")
                        nc.tensor.matmul(ps, etile, wcur[g], start=True, stop=True)
                        wn = wpool.tile([P2, GWS[g]], bf16, tag=f"w{g}")
                        nc.vector.tensor_mul(wn, ps, gt[(g, ci)][:, off, :])
                        wcur[g] = wn
                        if rep == 0:
                            if i == w - 1:
                                nc.sync.dma_start(out=snap_drams[g][:, 0, :], in_=wn)
                            if i == nstep - 1:
                                nc.sync.dma_start(out=snap_drams[g][:, 1, :], in_=wn)
                            if i == C - 2 and g == GRP_OF[0]:
                                nc.sync.dma_start(out=snap_drams[g][:, 2, :], in_=wn)

    nc.compile()
    _nc_cache[key] = nc
    return nc


def _build_timing(reps, w=W, nochain=False):
    """Timing-only module: the 38-step pass wrapped in a hardware For_i loop.
    Results are garbage (state carries across iterations); per-pass time =
    slope between two reps values. nochain=True makes matmuls read the initial
    w tile so the PE->DVE->PE roundtrip is broken (pure throughput measure)."""
    key = ("timing", w, reps, nochain)
    if key in _nc_cache:
        return _nc_cache[key]
    nstep = w + C
    nc = bacc.Bacc(
        "TRN2",
        target_bir_lowering=False,
        debug=False,
        enable_asserts=False,
        num_devices=NCORES,
    )
    bf16 = mybir.dt.bfloat16
    f32 = mybir.dt.float32
    e_dram = nc.dram_tensor("etil", [P2, P2], bf16, kind="ExternalInput").ap()
    g_drams = [
        nc.dram_tensor(f"gall{g}", [P2, nstep, GWS[g]], bf16,
                       kind="ExternalInput").ap()
        for g in range(NGRP)
    ]
    w_drams = [
        nc.dram_tensor(f"winit{g}", [P2, GWS[g]], bf16, kind="ExternalInput").ap()
        for g in range(NGRP)
    ]
    snap_drams = [
        nc.dram_tensor(f"snaps{g}", [P2, 3, GWS[g]], bf16,
                       kind="ExternalOutput").ap()
        for g in range(NGRP)
    ]
    chunks = []
    s0 = 0
    for cl in (2, 3, 4):          # small leading chunks: compute starts early
        if s0 < nstep:
            chunks.append((s0, min(cl, nstep - s0)))
            s0 += cl
    while s0 < nstep:
        chunks.append((s0, min(8, nstep - s0)))
        s0 += 8
    step_chunk = {}
    for ci, (c0, cl) in enumerate(chunks):
        for i in range(c0, c0 + cl):
            step_chunk[i] = (ci, i - c0)

    with tile.TileContext(nc) as tc:
        with ExitStack() as ctx:
            const = ctx.enter_context(tc.tile_pool(name="const", bufs=1))
            wpool = ctx.enter_context(tc.tile_pool(name="wp", bufs=4))
            gpool = ctx.enter_context(tc.tile_pool(name="gp", bufs=1))
            psum_p = ctx.enter_context(tc.tile_pool(name="ps", bufs=2, space="PSUM"))

            etile = const.tile([P2, P2], bf16)
            nc.sync.dma_start(out=etile, in_=e_dram)
            wcur = []
            for g in range(NGRP):
                wt = wpool.tile([P2, GWS[g]], bf16, tag=f"w{g}")
                nc.sync.dma_start(out=wt, in_=w_drams[g])
                wcur.append(wt)
            gt = {}
            for ci, (c0, cl) in enumerate(chunks):
                for g in range(NGRP):
                    t = gpool.tile([P2, cl, GWS[g]], bf16, tag=f"g{g}c{ci}")
                    nc.sync.dma_start(out=t, in_=g_drams[g][:, c0 : c0 + cl, :])
                    gt[(g, ci)] = t

            winit_tiles = list(wcur)
            with tc.For_i(0, reps, 1):
                for i in range(nstep):
                    ci, off = step_chunk[i]
                    for g in range(NGRP):
                        ps = psum_p.tile([P2, GWS[g]], f32, tag=f"ps{g}")
                        src = winit_tiles[g] if nochain else wcur[g]
                        nc.tensor.matmul(ps, etile, src, start=True, stop=True)
                        wn = wpool.tile([P2, GWS[g]], bf16, tag=f"w{g}")
                        nc.vector.tensor_mul(wn, ps, gt[(g, ci)][:, off, :])
                        wcur[g] = wn

            for g in range(NGRP):
                for slot in range(3):
                    nc.sync.dma_start(out=snap_drams[g][:, slot, :], in_=wcur[g])

    nc.compile()
    _nc_cache[key] = nc
    return nc


def _shifts_and_g(feats, seq_len, trans):
    mx = feats.max(axis=2)
    E64 = np.exp(trans.astype(np.float64)).T
    drift = []
    for b in range(6):
        fv = np.full(K, NEG, dtype=np.float64)
        fv[START] = 0.0
        for t in range(min(int(seq_len[b]), 256)):
            m = fv.max()
            wv = np.exp(fv - m)
            fv = np.log(E64.T @ wv + 1e-300) + m + feats[b, t]
            drift.append((fv.max() - m) - mx[b, t])
    mu = float(np.mean(drift))
    c = mx + mu
    Ccum = np.cumsum(c, axis=1, dtype=np.float64)
    C_at_L = Ccum[np.arange(B), seq_len - 1]

    taus = np.arange(1, T + 1)
    live = taus[None, :] <= seq_len[:, None]
    g = np.zeros((B, T, KS), dtype=np.float32)
    g[:, :, :KR] = np.exp(feats[:, :, :KR] - c[:, :, None]) * live[:, :, None]
    g[:, :, HOLD] = (taus[None, :] >= (seq_len[:, None] + 1)).astype(np.float32)
    return g, C_at_L


def _host_prep(feats, seq_len, trans, w=W):
    feats = np.ascontiguousarray(feats, dtype=np.float32)
    seq_len = np.asarray(seq_len, dtype=np.int64)
    trans = np.asarray(trans, dtype=np.float32)
    nstep = w + C

    g, C_at_L = _shifts_and_g(feats, seq_len, trans)

    Et = np.zeros((KS, KS), dtype=np.float32)
    Et[:KR, :KR] = np.exp(trans[:KR, :KR]).T
    Et[:KR, HOLD] = np.exp(trans[STOP, :KR])
    Et[HOLD, HOLD] = 1.0
    etil2 = np.zeros((P2, P2), dtype=np.float32)
    etil2[:KS, :KS] = Et
    etil2[KS:, KS:] = Et
    etil2 = etil2.astype(bfloat16)

    e_start = np.exp(trans[:KR, START])
    w1 = g[:, 0, :].copy()
    w1[:, :KR] *= e_start[None, :]
    w1[:, HOLD] = 0.0

    # sorted strided assignment: core k column c -> batch order[c*8+k]
    order = np.argsort(seq_len, kind="stable")
    asg = order.reshape(BC, NCORES)            # [c, core]

    # chain-step -> step table per (pair, half): 0-indexed into T axis, T=dead
    tmap = np.full((PAIRS, 2, nstep), T, dtype=np.int64)
    for j in range(PAIRS):
        for half in range(2):
            seg = 2 * j + half
            if seg == 0:
                for i in range(C - 1):
                    tmap[j, half, i] = 1 + i
            else:
                for i in range(nstep):
                    if i < w:
                        t = seg * C - w + 1 + i
                    else:
                        t = seg * C + 1 + (i - w)
                    tmap[j, half, i] = t - 1

    probe = np.zeros(KS, dtype=np.float32)
    probe[:KR] = 1.0 / KR

    gpad = np.concatenate([g, np.zeros((B, 1, KS), np.float32)], axis=1)
    galls, winits = [], []
    for cix in range(NCORES):
        bidx = asg[:, cix]                     # [BC] batch ids, ascending L
        sub = gpad[bidx]                       # [BC, T+1, KS]
        ga = [np.zeros((P2, nstep, GWS[gi]), dtype=np.float32) for gi in range(NGRP)]
        wi = [np.zeros((P2, GWS[gi]), dtype=np.float32) for gi in range(NGRP)]
        for j in range(PAIRS):
            gi, off, wd, lo = GRP_OF[j], POFF[j], WID[j], LO[j]
            for half in range(2):
                rows = slice(half * KS, (half + 1) * KS)
                # emis [wd_cols, nstep, KS] -> [KS, nstep, wd]
                em = sub[lo:, tmap[j, half], :]
                ga[gi][rows, :, off : off + wd] = em.transpose(2, 1, 0)
            wi[gi][0:KS, off : off + wd] = probe[:, None]
            wi[gi][KS:, off : off + wd] = probe[:, None]
            if j == 0:
                wi[gi][0:KS, off : off + wd] = w1[bidx[lo:]].T
        galls.append([np.ascontiguousarray(a.astype(bfloat16)) for a in ga])
        winits.append([np.ascontiguousarray(a.astype(bfloat16)) for a in wi])

    return etil2, galls, winits, C_at_L, g, w1, asg


def _exact_logZ(feats, seq_len, trans, b):
    E64 = np.exp(trans.astype(np.float64)).T
    fv = np.full(K, NEG, dtype=np.float64)
    fv[START] = 0.0
    for t in range(int(seq_len[b])):
        m = fv.max()
        wv = np.exp(fv - m)
        fv = np.log(E64.T @ wv + 1e-300) + m + feats[b, t].astype(np.float64)
    m = fv.max()
    return float(
        np.log(np.exp(fv - m + trans[STOP, :].astype(np.float64)).sum()) + m
    )


def _gold_score(feats, tags, seq_len, trans):
    feats = np.asarray(feats, dtype=np.float32)
    tags = np.asarray(tags, dtype=np.int64)
    seq_len = np.asarray(seq_len, dtype=np.int64)
    trans = np.asarray(trans, dtype=np.float32)
    tags_ext = np.concatenate(
        [np.full((B, 1), START, dtype=np.int64), tags], axis=1
    )
    trans_sc = trans[tags_ext[:, 1:], tags_ext[:, :-1]]
    emit_sc = np.take_along_axis(feats, tags_ext[:, 1:, None], axis=2)[..., 0]
    mask = np.arange(T)[None, :] < seq_len[:, None]
    last_tag = np.take_along_axis(tags_ext, seq_len[:, None], axis=1)[:, 0]
    return (
        np.where(mask, trans_sc + emit_sc, 0.0).sum(1, dtype=np.float64)
        + trans[STOP, last_tag]
    )


def _combine(snaps_list, feats, seq_len, trans, C_at_L, w1, asg):
    """snaps_list: per-core list of [94, 3, GWS[g]] arrays per group."""
    seq_len = np.asarray(seq_len, dtype=np.int64)
    trans = np.asarray(trans, dtype=np.float32)
    qs = np.zeros((S, B, KS), dtype=np.float64)
    rs = np.zeros((S, B, KS), dtype=np.float64)
    for cix in range(NCORES):
        bidx = asg[:, cix]
        sn = [np.asarray(a).astype(np.float32) for a in snaps_list[cix]]
        for j in range(PAIRS):
            gi, off, wd, lo = GRP_OF[j], POFF[j], WID[j], LO[j]
            cols = bidx[lo:]
            for half in range(2):
                seg = 2 * j + half
                rows = slice(half * KS, (half + 1) * KS)
                qs[seg, cols] = sn[gi][rows, 0, off : off + wd].T
                rs[seg, cols] = sn[gi][rows, 1, off : off + wd].T
            if j == 0:
                rs[0, cols] = sn[gi][0:KS, 2, off : off + wd].T
    qs[0] = w1.astype(np.float64)

    def n1(v):
        return v[..., :KR].sum(axis=-1)

    lk = np.zeros(B)
    logk = np.zeros((S, B))
    with np.errstate(divide="ignore", invalid="ignore"):
        for s in range(1, S):
            lk = lk + np.log(n1(rs[s - 1])) - np.log(n1(qs[s]))
            logk[s] = lk
    s_cap = np.minimum(seq_len // C, S - 1)
    cap = rs[s_cap, np.arange(B), HOLD]
    full = seq_len == T
    fdot = (
        rs[S - 1][:, :KR] * np.exp(trans[STOP, :KR].astype(np.float64))[None, :]
    ).sum(axis=1)
    cap = np.where(full, fdot, cap)
    with np.errstate(divide="ignore", invalid="ignore"):
        logZ = np.log(cap) + logk[s_cap, np.arange(B)] + C_at_L

    # staircase violations: column c of core k excluded from pair j although
    # its capture chain needs it (L >= 2jC). Sorted columns make this a
    # boundary check; recompute those elements exactly on the host.
    feats32 = np.asarray(feats, dtype=np.float32)
    patched = 0
    for cix in range(NCORES):
        bidx = asg[:, cix]
        L = seq_len[bidx]
        for j in range(PAIRS):
            lo = LO[j]
            bad = np.nonzero(L[:lo] >= 2 * j * C)[0]
            for c in bad:
                b = int(bidx[c])
                logZ[b] = _exact_logZ(feats32, seq_len, trans, b)
                patched += 1
    bad = ~np.isfinite(logZ)
    for b in np.nonzero(bad)[0]:
        logZ[b] = _exact_logZ(feats32, seq_len, trans, b)
        patched += 1
    return logZ, patched


def kernel(feats, tags, seq_len, transitions):
    feats = np.asarray(feats)
    etil2, galls, winits, C_at_L, g, w1, asg = _host_prep(
        feats, seq_len, transitions
    )
    nc = _build_module()
    in_maps = []
    for cix in range(NCORES):
        m = {"etil": etil2}
        for gi in range(NGRP):
            m[f"gall{gi}"] = galls[cix][gi]
            m[f"winit{gi}"] = winits[cix][gi]
        in_maps.append(m)
    res = run_bass_kernel_spmd(nc, in_maps, list(range(NCORES)))
    snaps_list = [
        [res.results[cix][f"snaps{gi}"] for gi in range(NGRP)]
        for cix in range(NCORES)
    ]
    logZ, _ = _combine(snaps_list, feats, seq_len, transitions, C_at_L, w1, asg)
    gold = _gold_score(feats, tags, seq_len, transitions)
    return np.float32(np.mean(logZ - gold))
